# revision 1
# baseline (speedup 1.0000x reference)
"""AttentionBlock (GroupNorm + single-head 4096x4096 attention + proj + residual)
on 8 Trainium2 NeuronCores.

Sharding: core c = 2*b + h handles image b (of 4), query-half h (of 2).
Each core:
  - transposes its image to channel-major (PE transposes, overlapped with
    GroupNorm statistics via bn_stats in the transposed layout),
  - computes kT [512,4096] and v [4096,512] for the full image,
  - computes qT for its 2048 query rows,
  - attention over its 2048 queries (softmax without max subtraction --
    scores are O(6) so exp is safe in fp32),
  - projection + bias + residual for its rows.
No collectives; k/v compute is duplicated across the half-pair (~10% FLOPs).

Precision: bf16 for all GEMMs (hn/q/k/v/scores/PV/proj), fp32 PSUM
accumulation, fp32 GroupNorm statistics, fp32 softmax row-sums /
normalization, fp32 residual.  Measured ~4e-4 max rel err vs fp32.
"""

import sys

sys.path.insert(0, "/opt/trn_rl_repo")

import numpy as np  # noqa: E402

import bass_rust  # noqa: E402
import concourse.bass as bass  # noqa: E402
import concourse.mybir as mybir  # noqa: E402
import concourse.tile as tile  # noqa: E402
from concourse.vector_clock import ScopedClock  # noqa: E402
from concourse.bass_utils import run_bass_kernel_spmd  # noqa: E402

F32 = mybir.dt.float32
F32R = mybir.dt.float32r
BF16 = mybir.dt.bfloat16
AF = mybir.ActivationFunctionType
OP = mybir.AluOpType
AX = mybir.AxisListType

B, H, W, C = 4, 64, 64, 512
HW = H * W            # 4096 positions per image
HALF = HW // 2        # 2048 query rows per core
GROUPS = 32
GSIZE = C // GROUPS   # 16 channels per group
EPS = 1e-5
SM_SCALE = 1.0 / float(np.sqrt(C))
N_CORES = 8
CT = C // 128         # 4 channel partition-tiles
JT = HW // 128        # 32 position partition-tiles
JC = HW // 512        # 8 position chunks (kT/v build)
QC = HALF // 512      # 4 query chunks (qT build)
IB = HALF // 512      # 4 query i-blocks (attention)


# --- workaround: walrus in this container rejects instructions carrying more
# than one sync-wait command.  Move extra waits onto same-engine NOPs placed
# immediately before the instruction (engine program order makes this exact).
def _split_multi_waits(nc, max_waits=1):
    n = 0
    for f in nc.m.functions:
        for bb in f.blocks:
            newlist = []
            for inst in bb.instructions:
                si = inst.sync_info
                waits = list(si.on_wait) if si is not None else []
                if len(waits) > max_waits:
                    n += 1
                    for k, wt in enumerate(waits[:-max_waits]):
                        nop = bass_rust.InstNoOp(
                            name=f"{inst.name}-sw{k}", engine=inst.engine)
                        nop.sync_info = mybir.SyncInfo(on_wait=[wt], on_update=[])
                        newlist.append(nop)
                    inst.sync_info = mybir.SyncInfo(
                        on_wait=waits[-max_waits:], on_update=list(si.on_update))
                newlist.append(inst)
            bb.instructions[:] = newlist
    return n


def _split_drain_and_barrier(self, tick_clock, wait_clock):
    # same as TileContext._drain_and_barrier but with the tail drain's waits
    # split onto single-wait NOPs (same walrus limitation as above).
    drain_inst = self.nc.sync.drain()
    wait_clock.add_sem_waits(
        drain_inst.ins, ScopedClock({None: tick_clock.global_clock}))
    mi = drain_inst.ins
    waits = list(mi.sync_info.on_wait) if mi.sync_info is not None else []
    if len(waits) > 1:
        mi.sync_info.on_wait = []
        for wt in waits:
            wi = self.nc.sync.nop(nofuse=True, hint="tail_drain_wait")
            wi.ins.sync_info = mybir.SyncInfo(on_wait=[wt], on_update=[])
    self.nc.all_engine_barrier()
    assert self.sems is not None
    popped = self.nc._tile_sem_poison_stack.pop()
    assert popped is self._sem_poison
    self.nc.clear_and_free_semaphores(list(self.sems.allocated().values()))
    self.nc.all_engine_barrier()


tile.TileContext._drain_and_barrier = _split_drain_and_barrier


def build_program():
    nc = bass.Bass()

    # xbh rows are permuted per-core so the query half is always rows
    # [0, HALF), and pre-split by channel partition-tile so each xbar
    # transpose reads a fully contiguous region.  Attention is
    # position-order invariant over keys/values.
    xbh = nc.dram_tensor("xbh", [CT, HW, 128], BF16, kind="ExternalInput")
    xq = nc.dram_tensor("xq", [HALF, C], F32, kind="ExternalInput")
    wq = nc.dram_tensor("wq", [C, C], F32, kind="ExternalInput")
    wk = nc.dram_tensor("wk", [C, C], F32, kind="ExternalInput")
    wv = nc.dram_tensor("wv", [C, C], F32, kind="ExternalInput")
    wp = nc.dram_tensor("wp", [C, C], F32, kind="ExternalInput")
    bqd = nc.dram_tensor("bq", [C, 1], F32, kind="ExternalInput")
    bkd = nc.dram_tensor("bk", [C, 1], F32, kind="ExternalInput")
    bvd = nc.dram_tensor("bv", [C], F32, kind="ExternalInput")
    bpd = nc.dram_tensor("bp", [C], F32, kind="ExternalInput")
    gamd = nc.dram_tensor("gamma", [C, 1], F32, kind="ExternalInput")
    betd = nc.dram_tensor("beta", [C, 1], F32, kind="ExternalInput")
    idnd = nc.dram_tensor("idn", [128, 128], BF16, kind="ExternalInput")
    gseld = nc.dram_tensor("gsel", [GROUPS, C], F32, kind="ExternalInput")
    # gsel2[p, ct, g] = 1/GSIZE where channel ct*128+p belongs to group g
    gsel2d = nc.dram_tensor("gsel2", [128, CT, GROUPS], F32, kind="ExternalInput")
    yd = nc.dram_tensor("y", [HALF, C], F32, kind="ExternalOutput")

    xqt = xq[:, :].rearrange("(t p) c -> t p c", p=128)   # [16,128,512]
    yt = yd[:, :].rearrange("(t p) c -> t p c", p=128)    # [16,128,512]

    with tile.TileContext(nc) as tc:
        # ---------------- persistent storage + constants ----------------
        store = tc.alloc_tile_pool(name="store", bufs=1)
        kT = store.tile([128, CT, HW], BF16)      # kT[c%128, c//128, j]
        vS = store.tile([128, JT, C], BF16)       # v[j%128, j//128, c]
        qT = store.tile([128, CT, HALF], BF16)    # qT[c%128, c//128, i]
        # x^T / xq^T in bf16, one tile per (channel-tile, image-half) so the
        # xbar-transpose DMAs and bn_stats pipeline
        xTs = [[store.tile([128, HALF], BF16, tag=f"xT{ct}_{hf}",
                           name=f"xT{ct}_{hf}") for hf in range(2)]
               for ct in range(CT)]
        wpr = store.tile([128, CT, C], BF16)      # wp cast, [cin%128, cin//128, cout]
        cst = tc.alloc_tile_pool(name="cst", bufs=1)
        idn = cst.tile([128, 128], BF16)
        nc.sync.dma_start(out=idn, in_=idnd[:, :])
        gsel = cst.tile([GROUPS, C], F32)
        nc.scalar.dma_start(out=gsel, in_=gseld[:, :])
        gsel2 = cst.tile([128, CT, GROUPS], F32)
        nc.scalar.dma_start(out=gsel2, in_=gsel2d[:, :, :])
        onesb = cst.tile([128, 1], BF16)
        nc.vector.memset(onesb, 1.0)
        ones4 = cst.tile([128, IB], F32)
        nc.vector.memset(ones4, 1.0)
        ones32 = cst.tile([GROUPS, 1], F32)
        nc.vector.memset(ones32, 1.0)
        # DRAM scratch to re-layout softmax row-sums [1,512] -> [128,4]
        sumscr = nc.dram_tensor("sumscr", [IB, 512], F32)
        bq_sb = cst.tile([128, CT], F32)
        bk_sb = cst.tile([128, CT], F32)
        gam_sb = cst.tile([128, CT], F32)
        bet_sb = cst.tile([128, CT], F32)
        for ct in range(CT):
            nc.scalar.dma_start(out=bq_sb[:, ct:ct + 1], in_=bqd[ct * 128:(ct + 1) * 128, :])
            nc.scalar.dma_start(out=bk_sb[:, ct:ct + 1], in_=bkd[ct * 128:(ct + 1) * 128, :])
            nc.scalar.dma_start(out=gam_sb[:, ct:ct + 1], in_=gamd[ct * 128:(ct + 1) * 128, :])
            nc.scalar.dma_start(out=bet_sb[:, ct:ct + 1], in_=betd[ct * 128:(ct + 1) * 128, :])
        bv_bc = cst.tile([128, C], F32)
        nc.scalar.dma_start(out=bv_bc, in_=bvd[:].partition_broadcast(128))
        bp_bc = cst.tile([128, C], F32)
        nc.scalar.dma_start(out=bp_bc, in_=bpd[:].partition_broadcast(128))
        s_sb = cst.tile([128, CT], F32)   # GN scale per channel
        t_sb = cst.tile([128, CT], F32)   # GN shift per channel

        # cast weights for q/k/v to bf16 (freed after phase B)
        wstage = tc.alloc_tile_pool(name="wstage", bufs=1)
        wrnd = tc.alloc_tile_pool(name="wrnd", bufs=1)
        wqr = wrnd.tile([128, CT, C], BF16)
        wkr = wrnd.tile([128, CT, C], BF16)
        wvr = wrnd.tile([128, CT, C], BF16)
        for wd, wr in ((wq, wqr), (wk, wkr), (wv, wvr), (wp, wpr)):
            stg = wstage.tile([128, CT, C], F32, tag="wstage")
            nc.scalar.dma_start(
                out=stg, in_=wd[:, :].rearrange("(t p) c -> p t c", p=128))
            nc.vector.tensor_copy(wr[:, :, :], stg[:, :, :])

        # -------- phase A: PE-transpose x (bf16) + GroupNorm stats ----------
        with tc.tile_pool(name="pa_sb", bufs=8) as pa, \
             tc.tile_pool(name="pa_tp", bufs=3, space="PSUM") as pa_tp, \
             tc.tile_pool(name="pa_ps", bufs=2, space="PSUM") as pa_ps, \
             tc.tile_pool(name="pa_small", bufs=1) as pas:
            stats_sb = pas.tile([128, CT, JC, 6], F32)
            for jc in range(JC):
                for ct in range(CT):
                    xn = pa.tile([128, 4, 128], BF16, tag="xn")
                    xeng = nc.sync if ct % 2 == 0 else nc.scalar
                    xeng.dma_start(
                        out=xn,
                        in_=xbh[ct, jc * 512:(jc + 1) * 512, :].rearrange(
                            "(a p) c -> p a c", p=128))
                    tp = pa_tp.tile([128, 4, 128], BF16, tag="tp")
                    for jt in range(4):
                        nc.tensor.transpose(tp[:, jt, :], xn[:, jt, :], idn[:, :])
                    nc.scalar.activation(
                        xTs[ct][jc // 4][:, (jc % 4) * 512:(jc % 4 + 1) * 512],
                        tp[:, :, :], AF.Copy)
                    nc.vector.bn_stats(
                        out=stats_sb[:, ct, jc, :],
                        in_=xTs[ct][jc // 4][:, (jc % 4) * 512:(jc % 4 + 1) * 512])

            # per-channel stats -> per-group mean / E[x^2] (batched)
            g2 = pa_ps.tile([GROUPS, 2], F32, tag="gagg")
            mv_all = pas.tile([128, CT, 2], F32)
            sp_all = pas.tile([128, CT, 2], F32)
            for ct in range(CT):
                nc.vector.bn_aggr(out=mv_all[:, ct, :], in_=stats_sb[:, ct, :, :])
            nc.vector.tensor_mul(sp_all[:, :, 0], mv_all[:, :, 0], mv_all[:, :, 0])
            nc.vector.tensor_add(sp_all[:, :, 1], sp_all[:, :, 0], mv_all[:, :, 1])
            nc.vector.tensor_copy(sp_all[:, :, 0], mv_all[:, :, 0])
            for ct in range(CT):
                nc.tensor.matmul(g2[:, :], gsel2[:, ct, :], sp_all[:, ct, :],
                                 start=(ct == 0), stop=(ct == CT - 1))
            # group mean/var -> (mean, rstd)
            mv2 = pas.tile([GROUPS, 2], F32)
            nc.scalar.activation(mv2[:, :], g2[:, :], AF.Copy)   # (mean, E[x^2])
            var = pas.tile([GROUPS, 1], F32)
            nc.vector.tensor_mul(var[:, :], mv2[:, 0:1], mv2[:, 0:1])
            nc.vector.tensor_sub(var[:, :], mv2[:, 1:2], var[:, :])
            epst = pas.tile([GROUPS, 1], F32)
            nc.vector.memset(epst, EPS)
            sd = pas.tile([GROUPS, 1], F32)
            nc.scalar.activation(sd[:, :], var[:, :], AF.Sqrt, bias=epst[:, :])
            nc.vector.reciprocal(mv2[:, 1:2], sd[:, :])
            # broadcast group (mean, rstd) to channels, then s/t (batched)
            bc_all = pas.tile([128, CT, 2], F32)
            for ct in range(CT):
                pbc = pa_ps.tile([128, 2], F32, tag="bcast")
                nc.tensor.matmul(pbc[:, :], gsel[:, ct * 128:(ct + 1) * 128],
                                 mv2[:, :], start=True, stop=True)
                nc.scalar.activation(bc_all[:, ct, :], pbc[:, :], AF.Copy)
            nc.vector.tensor_mul(s_sb[:, :], gam_sb[:, :], bc_all[:, :, 1])
            tmp = pas.tile([128, CT], F32)
            nc.vector.tensor_mul(tmp[:, :], bc_all[:, :, 0], s_sb[:, :])
            nc.vector.tensor_sub(t_sb[:, :], bet_sb[:, :], tmp[:, :])

        # ---------------- phase B: normalize + K,V (and Q) GEMMs ------------
        def qkv_chunk(pb, pb_ps, jc):
            hnT = pb.tile([128, CT, 512], BF16, tag="hnT")
            for ct in range(CT):
                # hnT = s * xT + t  (per-channel; channels on partitions)
                nc.vector.tensor_scalar(
                    hnT[:, ct, :],
                    xTs[ct][jc // 4][:, (jc % 4) * 512:(jc % 4 + 1) * 512],
                    s_sb[:, ct:ct + 1], t_sb[:, ct:ct + 1], OP.mult, OP.add)
            for ct in range(CT):
                pk = pb_ps.tile([128, 512], F32, tag="qkv")
                for k in range(CT):
                    nc.tensor.matmul(
                        pk[:, :], wkr[:, k, ct * 128:(ct + 1) * 128],
                        hnT[:, k, :], start=(k == 0), stop=(k == CT - 1))
                nc.scalar.activation(
                    kT[:, ct, jc * 512:(jc + 1) * 512], pk[:, :],
                    AF.Identity, bias=bk_sb[:, ct:ct + 1])
            if jc < QC:   # rows [0, HALF) are the query rows
                for ct in range(CT):
                    pq = pb_ps.tile([128, 512], F32, tag="qkv")
                    for k in range(CT):
                        nc.tensor.matmul(
                            pq[:, :], wqr[:, k, ct * 128:(ct + 1) * 128],
                            hnT[:, k, :], start=(k == 0), stop=(k == CT - 1))
                    nc.scalar.activation(
                        qT[:, ct, jc * 512:(jc + 1) * 512], pq[:, :],
                        AF.Identity, bias=bq_sb[:, ct:ct + 1])
            for jp in range(4):
                pv = pb_ps.tile([128, 512], F32, tag="qkv")
                for k in range(CT):
                    nc.tensor.matmul(
                        pv[:, :], hnT[:, k, jp * 128:(jp + 1) * 128],
                        wvr[:, k, :], start=(k == 0), stop=(k == CT - 1))
                nc.vector.tensor_tensor(
                    vS[:, jc * 4 + jp, :], pv[:, :], bv_bc[:, :], OP.add)

        with tc.tile_pool(name="pb_sb", bufs=3) as pb, \
             tc.tile_pool(name="pb_ps", bufs=6, space="PSUM") as pb_ps:
            for jc in range(JC):
                qkv_chunk(pb, pb_ps, jc)

        wrnd.release()    # free wq/wk/wv bf16 copies (LIFO with wstage)
        wstage.release()

        # ---------------- phase C: attention + projection + residual --------
        with tc.tile_pool(name="pc_sb", bufs=4) as pcs, \
             tc.tile_pool(name="pc_res", bufs=1) as pcr, \
             tc.tile_pool(name="pc_o", bufs=2) as pco, \
             tc.tile_pool(name="ps_s", bufs=2, space="PSUM") as ps_s, \
             tc.tile_pool(name="ps_o", bufs=1, space="PSUM") as ps_o, \
             tc.tile_pool(name="ps_r", bufs=1, space="PSUM") as ps_r, \
             tc.tile_pool(name="ps_y", bufs=1, space="PSUM") as ps_y:
            for ib in range(IB):
                po = ps_o.tile([128, CT, 512], F32)
                pr = ps_r.tile([1, 512], F32)
                # prefetch residual rows + bias for this i-block
                bpxs = []
                for ip in range(4):
                    xr = pcr.tile([128, C], F32, tag=f"xr{ip}")
                    nc.sync.dma_start(out=xr, in_=xqt[ib * 4 + ip, :, :])
                    bpx = pcr.tile([128, C], F32, tag=f"bpx{ip}")
                    nc.vector.tensor_tensor(bpx[:, :], xr[:, :], bp_bc[:, :], OP.add)
                    bpxs.append(bpx)
                for j in range(JT):
                    pss = ps_s.tile([128, 512], F32, tag="scores")
                    for k in range(CT):
                        nc.tensor.matmul(
                            pss[:, :], kT[:, k, j * 128:(j + 1) * 128],
                            qT[:, k, ib * 512:(ib + 1) * 512],
                            start=(k == 0), stop=(k == CT - 1))
                    et = pcs.tile([128, 512], BF16, tag="exp")
                    nc.scalar.activation(et[:, :], pss[:, :], AF.Exp, scale=SM_SCALE)
                    for ct in range(CT):
                        nc.tensor.matmul(
                            po[:, ct, :], vS[:, j, ct * 128:(ct + 1) * 128],
                            et[:, :], start=(j == 0), stop=(j == JT - 1))
                    # row-sums of exp: ones^T @ expT -> [1, 512] (i on free dim)
                    nc.tensor.matmul(
                        pr[:, :], onesb[:, :], et[:, :],
                        start=(j == 0), stop=(j == JT - 1))
                # move the row-sums into per-partition layout [128, 4] via a
                # DRAM bounce (off-engine), then one cheap elementwise divide
                srow = pcs.tile([1, 512], F32, tag="srow")
                nc.scalar.activation(srow[:, :], pr[:, :], AF.Copy)
                nc.sync.dma_start(out=sumscr[ib:ib + 1, :], in_=srow[:, :])
                st4 = pcr.tile([128, IB], F32, tag="st4")
                nc.gpsimd.dma_start(
                    out=st4[:, :],
                    in_=sumscr[ib, :].rearrange("(b p) -> p b", p=128))
                rt = pcr.tile([128, IB], F32, tag="rt")
                nc.vector.reciprocal(rt[:, :], st4[:, :])
                # unnormalized outT evicts on the (mostly idle) scalar engine
                ot = pco.tile([128, CT, 512], BF16, tag="outT")
                for ct in range(CT):
                    nc.scalar.activation(ot[:, ct, :], po[:, ct, :], AF.Copy)
                for ip in range(4):
                    py = ps_y.tile([128, 512], F32, tag="proj")
                    for ct in range(CT):
                        nc.tensor.matmul(
                            py[:, :], ot[:, ct, ip * 128:(ip + 1) * 128],
                            wpr[:, ct, :], start=(ct == 0), stop=(ct == CT - 1))
                    y2 = pcs.tile([128, C], F32, tag="y2")
                    nc.vector.scalar_tensor_tensor(
                        y2[:, :], py[:, :], rt[:, ip:ip + 1], bpxs[ip][:, :],
                        OP.mult, OP.add)
                    nc.sync.dma_start(out=yt[ib * 4 + ip, :, :], in_=y2[:, :])

        cst.release()
        store.release()

    _split_multi_waits(nc)
    return nc


_PROGRAM = None


def _get_program():
    global _PROGRAM
    if _PROGRAM is None:
        _PROGRAM = build_program()
    return _PROGRAM


def make_in_maps(x, gamma, beta, wq, bq, wk, bk, wv, bv, wp, bp):
    import ml_dtypes
    f32 = lambda a: np.ascontiguousarray(a, dtype=np.float32)
    xr = f32(x).reshape(B, HW, C)
    xr_bf = np.ascontiguousarray(xr.astype(ml_dtypes.bfloat16))
    gsel = np.zeros((GROUPS, C), dtype=np.float32)
    for g in range(GROUPS):
        gsel[g, g * GSIZE:(g + 1) * GSIZE] = 1.0
    gsel2 = np.zeros((128, CT, GROUPS), dtype=np.float32)
    for p in range(128):
        for ct in range(CT):
            gsel2[p, ct, (ct * 128 + p) // GSIZE] = 1.0 / GSIZE
    common = {
        "wq": f32(wq), "wk": f32(wk), "wv": f32(wv), "wp": f32(wp),
        "bq": f32(bq).reshape(C, 1), "bk": f32(bk).reshape(C, 1),
        "bv": f32(bv), "bp": f32(bp),
        "gamma": f32(gamma).reshape(C, 1), "beta": f32(beta).reshape(C, 1),
        "gsel": gsel, "gsel2": gsel2,
        "idn": np.eye(128, dtype=np.float32).astype(ml_dtypes.bfloat16),
    }
    in_maps = []
    for c in range(N_CORES):
        b, h = c // 2, c % 2
        m = dict(common)
        if h == 0:
            xp = xr_bf[b]
        else:
            xp = np.concatenate([xr_bf[b, HALF:], xr_bf[b, :HALF]], axis=0)
        # [HW, C] -> [CT, HW, 128] (channel-tile major, contiguous per tile)
        m["xbh"] = np.ascontiguousarray(
            xp.reshape(HW, CT, 128).transpose(1, 0, 2))
        m["xq"] = np.ascontiguousarray(xr[b, h * HALF:(h + 1) * HALF])
        in_maps.append(m)
    return in_maps


def kernel(x, gamma, beta, wq, bq, wk, bk, wv, bv, wp, bp, _trace=False):
    nc = _get_program()
    in_maps = make_in_maps(x, gamma, beta, wq, bq, wk, bk, wv, bv, wp, bp)
    res = run_bass_kernel_spmd(nc, in_maps, list(range(N_CORES)), trace=_trace)
    out = np.empty((B, HW, C), dtype=np.float32)
    for c in range(N_CORES):
        b, h = c // 2, c % 2
        out[b, h * HALF:(h + 1) * HALF] = res.results[c]["y"]
    if _trace:
        kernel._last_result = res
    return out.reshape(B, H, W, C)



# revision 9
# speedup vs baseline: 1.3566x; 1.3566x over previous
"""AttentionBlock (GroupNorm + single-head 4096x4096 attention + proj + residual)
on 8 Trainium2 NeuronCores.

Sharding: core c = 2*b + h handles image b (of 4), query-half h (of 2).
Each core:
  - receives x pre-transposed to channel-major bf16 (host does the transpose),
  - computes GroupNorm statistics via bn_stats while x streams in,
  - computes kT [512,4096] and v [4096,512] for the full image (k/v duplicated
    across the half-pair, ~10% extra FLOPs, no collectives),
  - computes qT for its 2048 query rows,
  - attention over its 2048 queries, projection + bias + residual for its rows.

Precision: fp8e4m3 (TRN 240-max) with MatmulPerfMode.DoubleRow for ALL GEMMs
(hn/q/k/v/scores/PV/rowsum/proj), fp32 PSUM accumulation, fp32 GroupNorm
statistics, fp32 softmax row-sums / normalization, fp32 residual.

fp8 scale bookkeeping:
  - wq/wk/wv/wp host-scaled x16 (keeps N(0, 1/sqrt(C)) weights out of the fp8
    subnormal range); biases bq/bk/bv host-scaled x16 to match.
  - qT/kT hold 16q/16k; scores PSUM = 256*q.k; exp scale folds the 1/256.
  - exp has bias -2.0 (max score 6.81 -> et max e^4.81=123 < 240 fp8 max);
    the e^-2 factor cancels between numerator and row-sum.
  - vS holds 16v; po PSUM = 16*sum(et*v); ot eviction scales by 1/128.
  - proj PSUM py = (po/128) @ (16 wp) = 2*sum(et*v)@wp; rowsum matmul uses a
    2.0-valued ones vector so rt = 1/pr = 1/(2*sum(et)) normalizes exactly.
"""

import sys

sys.path.insert(0, "/opt/trn_rl_repo")

import numpy as np  # noqa: E402

import bass_rust  # noqa: E402
import concourse.bass as bass  # noqa: E402
import concourse.mybir as mybir  # noqa: E402
import concourse.tile as tile  # noqa: E402
from concourse.vector_clock import ScopedClock  # noqa: E402
from concourse.bass_utils import run_bass_kernel_spmd  # noqa: E402

F32 = mybir.dt.float32
BF16 = mybir.dt.bfloat16
F8 = mybir.dt.float8e4
AF = mybir.ActivationFunctionType
OP = mybir.AluOpType
DR = mybir.MatmulPerfMode.DoubleRow

B, H, W, C = 4, 64, 64, 512
HW = H * W            # 4096 positions per image
HALF = HW // 2        # 2048 query rows per core
GROUPS = 32
GSIZE = C // GROUPS   # 16 channels per group
EPS = 1e-5
N_CORES = 8
CT = C // 128         # 4 channel partition-tiles
JT = HW // 128        # 32 position partition-tiles
JC = HW // 512        # 8 position chunks (kT/v build)
QC = HALF // 512      # 4 query chunks (qT build)
IB = HALF // 512      # 4 query i-blocks (attention)
WSC = 16.0            # host-side weight/bias scale (fp8 subnormal avoidance)
SM8 = 1.0 / (WSC * WSC * float(np.sqrt(C)))   # exp scale on (16q).(16k) psum
EXPB = -2.0           # exp bias: keeps et = e^(s-2) <= e^4.9 < 240 (fp8 max)
OTS = 1.0 / 128.0     # po -> ot eviction scale (fp8 range)
ONESV = 2.0           # rowsum weights: pr = 2*sum(et) so rt=1/pr normalizes
                      # py = (po/128)@(16wp) = 2*sum(et*v)@wp exactly


# --- workaround: walrus in this container rejects instructions carrying more
# than one sync-wait command.  Move extra waits onto same-engine NOPs placed
# immediately before the instruction (engine program order makes this exact).
def _split_multi_waits(nc, max_waits=1):
    n = 0
    for f in nc.m.functions:
        for bb in f.blocks:
            newlist = []
            for inst in bb.instructions:
                si = inst.sync_info
                waits = list(si.on_wait) if si is not None else []
                if len(waits) > max_waits:
                    n += 1
                    for k, wt in enumerate(waits[:-max_waits]):
                        nop = bass_rust.InstNoOp(
                            name=f"{inst.name}-sw{k}", engine=inst.engine)
                        nop.sync_info = mybir.SyncInfo(on_wait=[wt], on_update=[])
                        newlist.append(nop)
                    inst.sync_info = mybir.SyncInfo(
                        on_wait=waits[-max_waits:], on_update=list(si.on_update))
                newlist.append(inst)
            bb.instructions[:] = newlist
    return n


def _split_drain_and_barrier(self, tick_clock, wait_clock):
    # same as TileContext._drain_and_barrier but with the tail drain's waits
    # split onto single-wait NOPs (same walrus limitation as above).
    drain_inst = self.nc.sync.drain()
    wait_clock.add_sem_waits(
        drain_inst.ins, ScopedClock({None: tick_clock.global_clock}))
    mi = drain_inst.ins
    waits = list(mi.sync_info.on_wait) if mi.sync_info is not None else []
    if len(waits) > 1:
        mi.sync_info.on_wait = []
        for wt in waits:
            wi = self.nc.sync.nop(nofuse=True, hint="tail_drain_wait")
            wi.ins.sync_info = mybir.SyncInfo(on_wait=[wt], on_update=[])
    self.nc.all_engine_barrier()
    assert self.sems is not None
    popped = self.nc._tile_sem_poison_stack.pop()
    assert popped is self._sem_poison
    self.nc.clear_and_free_semaphores(list(self.sems.allocated().values()))
    self.nc.all_engine_barrier()


tile.TileContext._drain_and_barrier = _split_drain_and_barrier


def build_program(split_waits=True):
    nc = bass.Bass()

    # xT rows are permuted per-core so the query half is always positions
    # [0, HALF), host-transposed to [C, HW] bf16.  Attention is
    # position-order invariant over keys/values.
    xTd = nc.dram_tensor("xT", [C, HW], BF16, kind="ExternalInput")
    xq = nc.dram_tensor("xq", [HALF, C], F32, kind="ExternalInput")
    wq = nc.dram_tensor("wq", [C, C], BF16, kind="ExternalInput")
    wk = nc.dram_tensor("wk", [C, C], BF16, kind="ExternalInput")
    wv = nc.dram_tensor("wv", [C, C], BF16, kind="ExternalInput")
    wp = nc.dram_tensor("wp", [C, C], BF16, kind="ExternalInput")
    bqd = nc.dram_tensor("bq", [C, 1], F32, kind="ExternalInput")
    bkd = nc.dram_tensor("bk", [C, 1], F32, kind="ExternalInput")
    bvd = nc.dram_tensor("bv", [C], F32, kind="ExternalInput")
    bpd = nc.dram_tensor("bp", [C], F32, kind="ExternalInput")
    gamd = nc.dram_tensor("gamma", [C, 1], F32, kind="ExternalInput")
    betd = nc.dram_tensor("beta", [C, 1], F32, kind="ExternalInput")
    gseld = nc.dram_tensor("gsel", [GROUPS, C], F32, kind="ExternalInput")
    # gsel2[p, ct, g] = 1/GSIZE where channel ct*128+p belongs to group g
    gsel2d = nc.dram_tensor("gsel2", [128, CT, GROUPS], F32, kind="ExternalInput")
    yd = nc.dram_tensor("y", [HALF, C], F32, kind="ExternalOutput")

    xqt = xq[:, :].rearrange("(t p) c -> t p c", p=128)   # [16,128,512]
    yt = yd[:, :].rearrange("(t p) c -> t p c", p=128)    # [16,128,512]

    with tile.TileContext(nc) as tc:
        # ---------------- persistent storage + constants ----------------
        store = tc.alloc_tile_pool(name="store", bufs=1)
        kT = store.tile([128, CT, HW], F8)       # kT[c%128, c//128, j] = 16k
        vS = store.tile([128, JT, C], F8)        # v[j%128, j//128, c] = 16v
        qT = store.tile([128, CT, HALF], F8)     # qT[c%128, c//128, i] = 16q
        # x^T in bf16, one tile per (channel-tile, image-half)
        xTs = [[store.tile([128, HALF], BF16, tag=f"xT{ct}_{hf}",
                           name=f"xT{ct}_{hf}") for hf in range(2)]
               for ct in range(CT)]
        wpr = store.tile([128, CT, C], F8)       # 16*wp, [cin%128, cin//128, cout]
        cst = tc.alloc_tile_pool(name="cst", bufs=1)
        gsel = cst.tile([GROUPS, C], F32)
        nc.scalar.dma_start(out=gsel, in_=gseld[:, :])
        gsel2 = cst.tile([128, CT, GROUPS], F32)
        nc.scalar.dma_start(out=gsel2, in_=gsel2d[:, :, :])
        # [128, 2, 16] so the DoubleRow weight AP's pair-dim step is 16 bytes
        # (walrus s3_lw_dual_fp8_restrictions requires step % 16 == 0)
        ones2 = cst.tile([128, 2, 16], F8)
        nc.vector.memset(ones2, ONESV)
        expb = cst.tile([128, 1], F32)
        nc.vector.memset(expb, EXPB)
        # DRAM scratch to re-layout softmax row-sums [1,512] -> [128,4]
        sumscr = nc.dram_tensor("sumscr", [IB, 512], F32)
        bq_sb = cst.tile([128, CT], F32)
        bk_sb = cst.tile([128, CT], F32)
        gam_sb = cst.tile([128, CT], F32)
        bet_sb = cst.tile([128, CT], F32)
        for ct in range(CT):
            nc.scalar.dma_start(out=bq_sb[:, ct:ct + 1], in_=bqd[ct * 128:(ct + 1) * 128, :])
            nc.scalar.dma_start(out=bk_sb[:, ct:ct + 1], in_=bkd[ct * 128:(ct + 1) * 128, :])
            nc.scalar.dma_start(out=gam_sb[:, ct:ct + 1], in_=gamd[ct * 128:(ct + 1) * 128, :])
            nc.scalar.dma_start(out=bet_sb[:, ct:ct + 1], in_=betd[ct * 128:(ct + 1) * 128, :])
        bv_bc = cst.tile([128, C], F32)
        nc.scalar.dma_start(out=bv_bc, in_=bvd[:].partition_broadcast(128))
        bp_bc = cst.tile([128, C], F32)
        nc.scalar.dma_start(out=bp_bc, in_=bpd[:].partition_broadcast(128))
        s_sb = cst.tile([128, CT], F32)   # GN scale per channel
        t_sb = cst.tile([128, CT], F32)   # GN shift per channel

        # weight cast bf16 -> fp8 (x16 applied on host), DMA + cast on gpsimd
        # so the sync/scalar queues stay free for the x chunks
        wstage = tc.alloc_tile_pool(name="wstage", bufs=2)
        wrnd = tc.alloc_tile_pool(name="wrnd", bufs=1)
        wqr = wrnd.tile([128, CT, C], F8)
        wkr = wrnd.tile([128, CT, C], F8)
        wvr = wrnd.tile([128, CT, C], F8)
        for wd, wr in ((wq, wqr), (wk, wkr), (wv, wvr), (wp, wpr)):
            stg = wstage.tile([128, CT, C], BF16, tag="wstage")
            nc.gpsimd.dma_start(
                out=stg, in_=wd[:, :].rearrange("(t p) c -> p t c", p=128))
            nc.gpsimd.tensor_copy(wr[:, :, :], stg[:, :, :])

        # ------- phase A: stream xT in + GroupNorm stats (no PE work) -------
        with tc.tile_pool(name="pa_small", bufs=1) as pas:
            stats_sb = pas.tile([128, CT, JC, 6], F32)
            for jc in range(JC):
                hf, sc = jc // 4, (jc % 4) * 512
                for ct in range(CT):
                    xeng = nc.sync if ct % 2 == 0 else nc.scalar
                    xeng.dma_start(
                        out=xTs[ct][hf][:, sc:sc + 512],
                        in_=xTd[ct * 128:(ct + 1) * 128,
                                hf * HALF + sc:hf * HALF + sc + 512])
                    nc.vector.bn_stats(
                        out=stats_sb[:, ct, jc, :],
                        in_=xTs[ct][hf][:, sc:sc + 512])

            # per-channel stats -> per-group mean / E[x^2] (batched)
            with tc.tile_pool(name="pa_ps", bufs=2, space="PSUM") as pa_ps:
                g2 = pa_ps.tile([GROUPS, 2], F32, tag="gagg")
                mv_all = pas.tile([128, CT, 2], F32)
                sp_all = pas.tile([128, CT, 2], F32)
                for ct in range(CT):
                    nc.vector.bn_aggr(out=mv_all[:, ct, :], in_=stats_sb[:, ct, :, :])
                nc.vector.tensor_mul(sp_all[:, :, 0], mv_all[:, :, 0], mv_all[:, :, 0])
                nc.vector.tensor_add(sp_all[:, :, 1], sp_all[:, :, 0], mv_all[:, :, 1])
                nc.vector.tensor_copy(sp_all[:, :, 0], mv_all[:, :, 0])
                for ct in range(CT):
                    nc.tensor.matmul(g2[:, :], gsel2[:, ct, :], sp_all[:, ct, :],
                                     start=(ct == 0), stop=(ct == CT - 1))
                # group mean/var -> (mean, rstd)
                mv2 = pas.tile([GROUPS, 2], F32)
                nc.scalar.activation(mv2[:, :], g2[:, :], AF.Copy)   # (mean, E[x^2])
                var = pas.tile([GROUPS, 1], F32)
                nc.vector.tensor_mul(var[:, :], mv2[:, 0:1], mv2[:, 0:1])
                nc.vector.tensor_sub(var[:, :], mv2[:, 1:2], var[:, :])
                epst = pas.tile([GROUPS, 1], F32)
                nc.vector.memset(epst, EPS)
                sd = pas.tile([GROUPS, 1], F32)
                nc.scalar.activation(sd[:, :], var[:, :], AF.Sqrt, bias=epst[:, :])
                nc.vector.reciprocal(mv2[:, 1:2], sd[:, :])
                # broadcast group (mean, rstd) to channels, then s/t (batched)
                bc_all = pas.tile([128, CT, 2], F32)
                for ct in range(CT):
                    pbc = pa_ps.tile([128, 2], F32, tag="bcast")
                    nc.tensor.matmul(pbc[:, :], gsel[:, ct * 128:(ct + 1) * 128],
                                     mv2[:, :], start=True, stop=True)
                    nc.scalar.activation(bc_all[:, ct, :], pbc[:, :], AF.Copy)
                nc.vector.tensor_mul(s_sb[:, :], gam_sb[:, :], bc_all[:, :, 1])
                tmp = pas.tile([128, CT], F32)
                nc.vector.tensor_mul(tmp[:, :], bc_all[:, :, 0], s_sb[:, :])
                nc.vector.tensor_sub(t_sb[:, :], bet_sb[:, :], tmp[:, :])

        # ---------------- phase B: normalize + K,V (and Q) GEMMs ------------
        def qkv_chunk(pb, pb_ps, jc):
            hnT = pb.tile([128, CT, 512], F8, tag="hnT")
            for ct in range(CT):
                # hnT = s * xT + t  (per-channel; channels on partitions)
                eng = nc.gpsimd if ct % 2 == 0 else nc.vector
                eng.tensor_scalar(
                    hnT[:, ct, :],
                    xTs[ct][jc // 4][:, (jc % 4) * 512:(jc % 4 + 1) * 512],
                    s_sb[:, ct:ct + 1], t_sb[:, ct:ct + 1], OP.mult, OP.add)
            for ct in range(CT):
                pk = pb_ps.tile([128, 512], F32, tag="qkv")
                for k2 in range(2):
                    nc.tensor.matmul(
                        pk[:, :], wkr[:, 2 * k2:2 * k2 + 2, ct * 128:(ct + 1) * 128],
                        hnT[:, 2 * k2:2 * k2 + 2, :],
                        start=(k2 == 0), stop=(k2 == 1), perf_mode=DR)
                nc.scalar.activation(
                    kT[:, ct, jc * 512:(jc + 1) * 512], pk[:, :],
                    AF.Identity, bias=bk_sb[:, ct:ct + 1])
            if jc < QC:   # rows [0, HALF) are the query rows
                for ct in range(CT):
                    pq = pb_ps.tile([128, 512], F32, tag="qkv")
                    for k2 in range(2):
                        nc.tensor.matmul(
                            pq[:, :], wqr[:, 2 * k2:2 * k2 + 2, ct * 128:(ct + 1) * 128],
                            hnT[:, 2 * k2:2 * k2 + 2, :],
                            start=(k2 == 0), stop=(k2 == 1), perf_mode=DR)
                    nc.scalar.activation(
                        qT[:, ct, jc * 512:(jc + 1) * 512], pq[:, :],
                        AF.Identity, bias=bq_sb[:, ct:ct + 1])
            for jp in range(4):
                pv = pb_ps.tile([128, 512], F32, tag="qkv")
                for k2 in range(2):
                    nc.tensor.matmul(
                        pv[:, :], hnT[:, 2 * k2:2 * k2 + 2, jp * 128:(jp + 1) * 128],
                        wvr[:, 2 * k2:2 * k2 + 2, :],
                        start=(k2 == 0), stop=(k2 == 1), perf_mode=DR)
                nc.vector.tensor_tensor(
                    vS[:, jc * 4 + jp, :], pv[:, :], bv_bc[:, :], OP.add)

        with tc.tile_pool(name="pb_sb", bufs=3) as pb, \
             tc.tile_pool(name="pb_ps", bufs=6, space="PSUM") as pb_ps:
            for jc in range(JC):
                qkv_chunk(pb, pb_ps, jc)

        wrnd.release()    # free wq/wk/wv fp8 copies (LIFO with wstage)
        wstage.release()

        # ---------------- phase C: attention + projection + residual --------
        with tc.tile_pool(name="pc_sb", bufs=4) as pcs, \
             tc.tile_pool(name="pc_res", bufs=1) as pcr, \
             tc.tile_pool(name="pc_o", bufs=2) as pco, \
             tc.tile_pool(name="ps_s", bufs=2, space="PSUM") as ps_s, \
             tc.tile_pool(name="ps_o", bufs=1, space="PSUM") as ps_o, \
             tc.tile_pool(name="ps_r", bufs=1, space="PSUM") as ps_r, \
             tc.tile_pool(name="ps_y", bufs=1, space="PSUM") as ps_y:
            for ib in range(IB):
                po = ps_o.tile([128, CT, 512], F32)
                pr = ps_r.tile([1, 512], F32)
                # prefetch residual rows + bias for this i-block
                bpxs = []
                for ip in range(4):
                    xr = pcr.tile([128, C], F32, tag=f"xr{ip}")
                    nc.sync.dma_start(out=xr, in_=xqt[ib * 4 + ip, :, :])
                    bpx = pcr.tile([128, C], F32, tag=f"bpx{ip}")
                    nc.gpsimd.tensor_tensor(bpx[:, :], xr[:, :], bp_bc[:, :], OP.add)
                    bpxs.append(bpx)
                for j2 in range(JT // 2):
                    et = pcs.tile([128, 2, 512], F8, tag="exp")
                    for par in range(2):
                        j = 2 * j2 + par
                        pss = ps_s.tile([128, 512], F32, tag="scores")
                        for k2 in range(2):
                            nc.tensor.matmul(
                                pss[:, :], kT[:, 2 * k2:2 * k2 + 2, j * 128:(j + 1) * 128],
                                qT[:, 2 * k2:2 * k2 + 2, ib * 512:(ib + 1) * 512],
                                start=(k2 == 0), stop=(k2 == 1), perf_mode=DR)
                        nc.scalar.activation(et[:, par, :], pss[:, :], AF.Exp,
                                             bias=expb[:, :], scale=SM8)
                    for ct in range(CT):
                        nc.tensor.matmul(
                            po[:, ct, :], vS[:, 2 * j2:2 * j2 + 2, ct * 128:(ct + 1) * 128],
                            et[:, :, :], start=(j2 == 0), stop=(j2 == JT // 2 - 1),
                            perf_mode=DR)
                    # row-sums of exp: (2.0)^T @ etT -> [1, 512] (i on free dim)
                    nc.tensor.matmul(
                        pr[:, :], ones2[:, :, 0:1], et[:, :, :],
                        start=(j2 == 0), stop=(j2 == JT // 2 - 1), perf_mode=DR)
                # move the row-sums into per-partition layout [128, 4] via a
                # DRAM bounce (off-engine), then one cheap elementwise divide
                srow = pcs.tile([1, 512], F32, tag="srow")
                nc.scalar.activation(srow[:, :], pr[:, :], AF.Copy)
                nc.gpsimd.dma_start(out=sumscr[ib:ib + 1, :], in_=srow[:, :])
                st4 = pcr.tile([128, IB], F32, tag="st4")
                nc.gpsimd.dma_start(
                    out=st4[:, :],
                    in_=sumscr[ib, :].rearrange("(b p) -> p b", p=128))
                rt = pcr.tile([128, IB], F32, tag="rt")
                nc.vector.reciprocal(rt[:, :], st4[:, :])
                # unnormalized outT evicts on the scalar engine (scaled to fp8)
                ot = pco.tile([128, CT, 512], F8, tag="outT")
                for ct in range(CT):
                    nc.scalar.activation(ot[:, ct, :], po[:, ct, :], AF.Copy,
                                         scale=OTS)
                for ip in range(4):
                    py = ps_y.tile([128, 512], F32, tag="proj")
                    for c2 in range(2):
                        nc.tensor.matmul(
                            py[:, :], ot[:, 2 * c2:2 * c2 + 2, ip * 128:(ip + 1) * 128],
                            wpr[:, 2 * c2:2 * c2 + 2, :],
                            start=(c2 == 0), stop=(c2 == 1), perf_mode=DR)
                    y2 = pcs.tile([128, C], F32, tag="y2")
                    nc.vector.scalar_tensor_tensor(
                        y2[:, :], py[:, :], rt[:, ip:ip + 1], bpxs[ip][:, :],
                        OP.mult, OP.add)
                    nc.sync.dma_start(out=yt[ib * 4 + ip, :, :], in_=y2[:, :])

        cst.release()
        store.release()

    if split_waits:
        _split_multi_waits(nc)
    return nc


_PROGRAM = None


def _get_program():
    global _PROGRAM
    if _PROGRAM is None:
        _PROGRAM = build_program()
    return _PROGRAM


def make_in_maps(x, gamma, beta, wq, bq, wk, bk, wv, bv, wp, bp):
    import ml_dtypes
    f32 = lambda a: np.ascontiguousarray(a, dtype=np.float32)
    bf16 = lambda a: np.ascontiguousarray(np.asarray(a, dtype=np.float32).astype(ml_dtypes.bfloat16))
    xr = f32(x).reshape(B, HW, C)
    gsel = np.zeros((GROUPS, C), dtype=np.float32)
    for g in range(GROUPS):
        gsel[g, g * GSIZE:(g + 1) * GSIZE] = 1.0
    gsel2 = np.zeros((128, CT, GROUPS), dtype=np.float32)
    for p in range(128):
        for ct in range(CT):
            gsel2[p, ct, (ct * 128 + p) // GSIZE] = 1.0 / GSIZE
    common = {
        "wq": bf16(f32(wq) * WSC), "wk": bf16(f32(wk) * WSC),
        "wv": bf16(f32(wv) * WSC), "wp": bf16(f32(wp) * WSC),
        "bq": f32(bq).reshape(C, 1) * WSC, "bk": f32(bk).reshape(C, 1) * WSC,
        "bv": f32(bv) * WSC, "bp": f32(bp),
        "gamma": f32(gamma).reshape(C, 1), "beta": f32(beta).reshape(C, 1),
        "gsel": gsel, "gsel2": gsel2,
    }
    in_maps = []
    for c in range(N_CORES):
        b, h = c // 2, c % 2
        m = dict(common)
        if h == 0:
            xp = xr[b]
        else:
            xp = np.concatenate([xr[b, HALF:], xr[b, :HALF]], axis=0)
        # pre-transpose to channel-major [C, HW] bf16 on the host
        m["xT"] = np.ascontiguousarray(
            xp.T.astype(ml_dtypes.bfloat16))
        m["xq"] = np.ascontiguousarray(xr[b, h * HALF:(h + 1) * HALF])
        in_maps.append(m)
    return in_maps


def kernel(x, gamma, beta, wq, bq, wk, bk, wv, bv, wp, bp, _trace=False):
    nc = _get_program()
    in_maps = make_in_maps(x, gamma, beta, wq, bq, wk, bk, wv, bv, wp, bp)
    res = run_bass_kernel_spmd(nc, in_maps, list(range(N_CORES)), trace=_trace)
    out = np.empty((B, HW, C), dtype=np.float32)
    for c in range(N_CORES):
        b, h = c // 2, c % 2
        out[b, h * HALF:(h + 1) * HALF] = res.results[c]["y"]
    if _trace:
        kernel._last_result = res
    return out.reshape(B, H, W, C)


# revision 19
# speedup vs baseline: 1.3833x; 1.0197x over previous
"""AttentionBlock (GroupNorm + single-head 4096x4096 attention + proj + residual)
on 8 Trainium2 NeuronCores.

Sharding: core c = 2*b + h handles image b (of 4), query-half h (of 2).
Each core:
  - receives x pre-transposed to channel-major bf16 (host does the transpose),
  - computes GroupNorm statistics via bn_stats while x streams in,
  - computes kT [512,4096] and v [4096,512] for the full image (k/v duplicated
    across the half-pair, ~10% extra FLOPs, no collectives),
  - computes qT for its 2048 query rows,
  - attention over its 2048 queries, projection + bias + residual for its rows.

Precision: fp8e4m3 (TRN 240-max) with MatmulPerfMode.DoubleRow for ALL GEMMs
(hn/q/k/v/scores/PV/rowsum/proj), fp32 PSUM accumulation, fp32 GroupNorm
statistics, fp32 softmax row-sums / normalization, fp32 residual.

fp8 scale bookkeeping:
  - wq/wk/wv/wp host-scaled x16 (keeps N(0, 1/sqrt(C)) weights out of the fp8
    subnormal range); biases bq/bk/bv host-scaled x16 to match.
  - qT/kT hold 16q/16k; scores PSUM = 256*q.k; exp scale folds the 1/256.
  - exp has bias -2.0 (max score 6.81 -> et max e^4.81=123 < 240 fp8 max);
    the e^-2 factor cancels between numerator and row-sum.
  - vS holds 16v; po PSUM = 16*sum(et*v); ot eviction scales by 1/128.
  - proj PSUM py = (po/128) @ (16 wp) = 2*sum(et*v)@wp; rowsum matmul uses a
    2.0-valued ones vector so rt = 1/pr = 1/(2*sum(et)) normalizes exactly.
"""

import sys

sys.path.insert(0, "/opt/trn_rl_repo")

import numpy as np  # noqa: E402

import bass_rust  # noqa: E402
import concourse.bass as bass  # noqa: E402
import concourse.mybir as mybir  # noqa: E402
import concourse.tile as tile  # noqa: E402
from concourse.vector_clock import ScopedClock  # noqa: E402
from concourse.bass_utils import run_bass_kernel_spmd  # noqa: E402

F32 = mybir.dt.float32
BF16 = mybir.dt.bfloat16
F8 = mybir.dt.float8e4
AF = mybir.ActivationFunctionType
OP = mybir.AluOpType
DR = mybir.MatmulPerfMode.DoubleRow

B, H, W, C = 4, 64, 64, 512
HW = H * W            # 4096 positions per image
HALF = HW // 2        # 2048 query rows per core
GROUPS = 32
GSIZE = C // GROUPS   # 16 channels per group
EPS = 1e-5
N_CORES = 8
CT = C // 128         # 4 channel partition-tiles
JT = HW // 128        # 32 position partition-tiles
JC = HW // 512        # 8 position chunks (kT/v build)
QC = HALF // 512      # 4 query chunks (qT build)
IB = HALF // 512      # 4 query i-blocks (attention)
WSC = 16.0            # host-side weight/bias scale (fp8 subnormal avoidance)
SM8 = 1.0 / (WSC * WSC * float(np.sqrt(C)))   # exp scale on (16q).(16k) psum
EXPB = -2.0           # exp bias: keeps et = e^(s-2) <= e^4.9 < 240 (fp8 max)
OTS = 1.0 / 128.0     # po -> ot eviction scale (fp8 range)
ONESV = 2.0           # rowsum weights: pr = 2*sum(et) so rt=1/pr normalizes
                      # py = (po/128)@(16wp) = 2*sum(et*v)@wp exactly


# --- workaround: walrus in this container rejects instructions carrying more
# than one sync-wait command.  Move extra waits onto same-engine NOPs placed
# immediately before the instruction (engine program order makes this exact).
def _split_multi_waits(nc, max_waits=1):
    n = 0
    for f in nc.m.functions:
        for bb in f.blocks:
            newlist = []
            for inst in bb.instructions:
                si = inst.sync_info
                waits = list(si.on_wait) if si is not None else []
                if len(waits) > max_waits:
                    n += 1
                    for k, wt in enumerate(waits[:-max_waits]):
                        nop = bass_rust.InstNoOp(
                            name=f"{inst.name}-sw{k}", engine=inst.engine)
                        nop.sync_info = mybir.SyncInfo(on_wait=[wt], on_update=[])
                        newlist.append(nop)
                    inst.sync_info = mybir.SyncInfo(
                        on_wait=waits[-max_waits:], on_update=list(si.on_update))
                newlist.append(inst)
            bb.instructions[:] = newlist
    return n


def _split_drain_and_barrier(self, tick_clock, wait_clock):
    # same as TileContext._drain_and_barrier but with the tail drain's waits
    # split onto single-wait NOPs (same walrus limitation as above).
    drain_inst = self.nc.sync.drain()
    wait_clock.add_sem_waits(
        drain_inst.ins, ScopedClock({None: tick_clock.global_clock}))
    mi = drain_inst.ins
    waits = list(mi.sync_info.on_wait) if mi.sync_info is not None else []
    if len(waits) > 1:
        mi.sync_info.on_wait = []
        for wt in waits:
            wi = self.nc.sync.nop(nofuse=True, hint="tail_drain_wait")
            wi.ins.sync_info = mybir.SyncInfo(on_wait=[wt], on_update=[])
    self.nc.all_engine_barrier()
    assert self.sems is not None
    popped = self.nc._tile_sem_poison_stack.pop()
    assert popped is self._sem_poison
    self.nc.clear_and_free_semaphores(list(self.sems.allocated().values()))
    self.nc.all_engine_barrier()


tile.TileContext._drain_and_barrier = _split_drain_and_barrier


def build_program(split_waits=True):
    nc = bass.Bass()

    # xT rows are permuted per-core so the query half is always positions
    # [0, HALF), host-transposed to channel-major and chunk-tiled
    # [JC, CT, 128, 512] bf16 so every chunk DMA is one contiguous 128KB read.
    xTd = nc.dram_tensor("xT", [JC, CT, 128, 512], BF16, kind="ExternalInput")
    xq = nc.dram_tensor("xq", [HALF, C], F32, kind="ExternalInput")
    wq = nc.dram_tensor("wq", [C, C], BF16, kind="ExternalInput")
    wk = nc.dram_tensor("wk", [C, C], BF16, kind="ExternalInput")
    wv = nc.dram_tensor("wv", [C, C], BF16, kind="ExternalInput")
    wp = nc.dram_tensor("wp", [C, C], BF16, kind="ExternalInput")
    bqd = nc.dram_tensor("bq", [C, 1], F32, kind="ExternalInput")
    bkd = nc.dram_tensor("bk", [C, 1], F32, kind="ExternalInput")
    # bp here is host-computed bp + bv @ wp (bv folded through the attention)
    bpd = nc.dram_tensor("bp", [C], F32, kind="ExternalInput")
    gamd = nc.dram_tensor("gamma", [C, 1], F32, kind="ExternalInput")
    betd = nc.dram_tensor("beta", [C, 1], F32, kind="ExternalInput")
    gseld = nc.dram_tensor("gsel", [GROUPS, C], F32, kind="ExternalInput")
    # gsel2[p, ct, g] = 1/GSIZE where channel ct*128+p belongs to group g
    gsel2d = nc.dram_tensor("gsel2", [128, CT, GROUPS], F32, kind="ExternalInput")
    yd = nc.dram_tensor("y", [HALF, C], F32, kind="ExternalOutput")

    xqt = xq[:, :].rearrange("(t p) c -> t p c", p=128)   # [16,128,512]
    yt = yd[:, :].rearrange("(t p) c -> t p c", p=128)    # [16,128,512]

    with tile.TileContext(nc) as tc:
        # ---------------- persistent storage + constants ----------------
        store = tc.alloc_tile_pool(name="store", bufs=1)
        kT = store.tile([128, CT, HW], F8)       # kT[c%128, c//128, j] = 16k
        vS = store.tile([128, JT, C], F8)        # v[j%128, j//128, c] = 16v
        qT = store.tile([128, CT, HALF], F8)     # qT[c%128, c//128, i] = 16q
        # x^T in bf16, one tile per (channel-tile, image-half)
        xTs = [[store.tile([128, HALF], BF16, tag=f"xT{ct}_{hf}",
                           name=f"xT{ct}_{hf}") for hf in range(2)]
               for ct in range(CT)]
        wpr = store.tile([128, CT, C], F8)       # 16*wp, [cin%128, cin//128, cout]
        cst = tc.alloc_tile_pool(name="cst", bufs=1)
        gsel = cst.tile([GROUPS, C], F32)
        nc.scalar.dma_start(out=gsel, in_=gseld[:, :])
        gsel2 = cst.tile([128, CT, GROUPS], F32)
        nc.scalar.dma_start(out=gsel2, in_=gsel2d[:, :, :])
        # [128, 2, 16] so the DoubleRow weight AP's pair-dim step is 16 bytes
        # (walrus s3_lw_dual_fp8_restrictions requires step % 16 == 0)
        ones2 = cst.tile([128, 2, 16], F8)
        nc.vector.memset(ones2, ONESV)
        expb = cst.tile([128, 1], F32)
        nc.vector.memset(expb, EXPB)
        # DRAM scratch to re-layout softmax row-sums [1,512] -> [128,4]
        sumscr = nc.dram_tensor("sumscr", [IB, 512], F32)
        bq_sb = cst.tile([128, CT], F32)
        bk_sb = cst.tile([128, CT], F32)
        gam_sb = cst.tile([128, CT], F32)
        bet_sb = cst.tile([128, CT], F32)
        for ct in range(CT):
            nc.scalar.dma_start(out=bq_sb[:, ct:ct + 1], in_=bqd[ct * 128:(ct + 1) * 128, :])
            nc.scalar.dma_start(out=bk_sb[:, ct:ct + 1], in_=bkd[ct * 128:(ct + 1) * 128, :])
            nc.scalar.dma_start(out=gam_sb[:, ct:ct + 1], in_=gamd[ct * 128:(ct + 1) * 128, :])
            nc.scalar.dma_start(out=bet_sb[:, ct:ct + 1], in_=betd[ct * 128:(ct + 1) * 128, :])
        bp_bc = cst.tile([128, C], F32)
        nc.scalar.dma_start(out=bp_bc, in_=bpd[:].partition_broadcast(128))
        s_sb = cst.tile([128, CT], F32)   # GN scale per channel
        t_sb = cst.tile([128, CT], F32)   # GN shift per channel

        # weight cast bf16 -> fp8 (x16 applied on host); DMA on gpsimd queue so
        # the sync/scalar queues stay free for x chunks; cast on the (fast)
        # scalar engine -- Pool runs fp8 casts ~5x below spec.
        wstage = tc.alloc_tile_pool(name="wstage", bufs=2)
        wrnd = tc.alloc_tile_pool(name="wrnd", bufs=1)
        wqr = wrnd.tile([128, CT, C], F8)
        wkr = wrnd.tile([128, CT, C], F8)
        wvr = wrnd.tile([128, CT, C], F8)
        for wd, wr in ((wq, wqr), (wk, wkr), (wv, wvr), (wp, wpr)):
            stg = wstage.tile([128, CT, C], BF16, tag="wstage")
            nc.gpsimd.dma_start(
                out=stg, in_=wd[:, :].rearrange("(t p) c -> p t c", p=128))
            nc.scalar.activation(wr[:, :, :], stg[:, :, :], AF.Copy)

        # ------- phase A: stream xT in + GroupNorm stats (no PE work) -------
        with tc.tile_pool(name="pa_small", bufs=1) as pas:
            stats_sb = pas.tile([128, CT, JC, 6], F32)
            for jc in range(JC):
                hf, sc = jc // 4, (jc % 4) * 512
                for ct in range(CT):
                    xeng = nc.sync if ct % 2 == 0 else nc.scalar
                    xeng.dma_start(
                        out=xTs[ct][hf][:, sc:sc + 512],
                        in_=xTd[jc, ct, :, :])
                    nc.vector.bn_stats(
                        out=stats_sb[:, ct, jc, :],
                        in_=xTs[ct][hf][:, sc:sc + 512])

            # per-channel stats -> per-group mean / E[x^2] (batched)
            with tc.tile_pool(name="pa_ps", bufs=2, space="PSUM") as pa_ps:
                g2 = pa_ps.tile([GROUPS, 2], F32, tag="gagg")
                mv_all = pas.tile([128, CT, 2], F32)
                sp_all = pas.tile([128, CT, 2], F32)
                for ct in range(CT):
                    nc.vector.bn_aggr(out=mv_all[:, ct, :], in_=stats_sb[:, ct, :, :])
                nc.vector.tensor_mul(sp_all[:, :, 0], mv_all[:, :, 0], mv_all[:, :, 0])
                nc.vector.tensor_add(sp_all[:, :, 1], sp_all[:, :, 0], mv_all[:, :, 1])
                nc.vector.tensor_copy(sp_all[:, :, 0], mv_all[:, :, 0])
                for ct in range(CT):
                    nc.tensor.matmul(g2[:, :], gsel2[:, ct, :], sp_all[:, ct, :],
                                     start=(ct == 0), stop=(ct == CT - 1))
                # group mean/var -> (mean, rstd)
                mv2 = pas.tile([GROUPS, 2], F32)
                nc.scalar.activation(mv2[:, :], g2[:, :], AF.Copy)   # (mean, E[x^2])
                var = pas.tile([GROUPS, 1], F32)
                nc.vector.tensor_mul(var[:, :], mv2[:, 0:1], mv2[:, 0:1])
                nc.vector.tensor_sub(var[:, :], mv2[:, 1:2], var[:, :])
                epst = pas.tile([GROUPS, 1], F32)
                nc.vector.memset(epst, EPS)
                sd = pas.tile([GROUPS, 1], F32)
                nc.scalar.activation(sd[:, :], var[:, :], AF.Sqrt, bias=epst[:, :])
                nc.vector.reciprocal(mv2[:, 1:2], sd[:, :])
                # broadcast group (mean, rstd) to channels, then s/t (batched)
                bc_all = pas.tile([128, CT, 2], F32)
                for ct in range(CT):
                    pbc = pa_ps.tile([128, 2], F32, tag="bcast")
                    nc.tensor.matmul(pbc[:, :], gsel[:, ct * 128:(ct + 1) * 128],
                                     mv2[:, :], start=True, stop=True)
                    nc.scalar.activation(bc_all[:, ct, :], pbc[:, :], AF.Copy)
                nc.vector.tensor_mul(s_sb[:, :], gam_sb[:, :], bc_all[:, :, 1])
                tmp = pas.tile([128, CT], F32)
                nc.vector.tensor_mul(tmp[:, :], bc_all[:, :, 0], s_sb[:, :])
                nc.vector.tensor_sub(t_sb[:, :], bet_sb[:, :], tmp[:, :])

        # ---------------- phase B: normalize + K,V (and Q) GEMMs ------------
        def qkv_chunk(pb, pb_ps, jc):
            hnT = pb.tile([128, CT, 512], F8, tag="hnT")
            for ct in range(CT):
                # hnT = s * xT + t  (per-channel; channels on partitions)
                nc.vector.tensor_scalar(
                    hnT[:, ct, :],
                    xTs[ct][jc // 4][:, (jc % 4) * 512:(jc % 4 + 1) * 512],
                    s_sb[:, ct:ct + 1], t_sb[:, ct:ct + 1], OP.mult, OP.add)
            for ct in range(CT):
                pk = pb_ps.tile([128, 512], F32, tag="qkv")
                for k2 in range(2):
                    nc.tensor.matmul(
                        pk[:, :], wkr[:, 2 * k2:2 * k2 + 2, ct * 128:(ct + 1) * 128],
                        hnT[:, 2 * k2:2 * k2 + 2, :],
                        start=(k2 == 0), stop=(k2 == 1), perf_mode=DR)
                nc.scalar.activation(
                    kT[:, ct, jc * 512:(jc + 1) * 512], pk[:, :],
                    AF.Identity, bias=bk_sb[:, ct:ct + 1])
            if jc < QC:   # rows [0, HALF) are the query rows
                for ct in range(CT):
                    pq = pb_ps.tile([128, 512], F32, tag="qkv")
                    for k2 in range(2):
                        nc.tensor.matmul(
                            pq[:, :], wqr[:, 2 * k2:2 * k2 + 2, ct * 128:(ct + 1) * 128],
                            hnT[:, 2 * k2:2 * k2 + 2, :],
                            start=(k2 == 0), stop=(k2 == 1), perf_mode=DR)
                    nc.scalar.activation(
                        qT[:, ct, jc * 512:(jc + 1) * 512], pq[:, :],
                        AF.Identity, bias=bq_sb[:, ct:ct + 1])
            for jp in range(4):
                pv = pb_ps.tile([128, 512], F32, tag="qkv")
                for k2 in range(2):
                    nc.tensor.matmul(
                        pv[:, :], hnT[:, 2 * k2:2 * k2 + 2, jp * 128:(jp + 1) * 128],
                        wvr[:, 2 * k2:2 * k2 + 2, :],
                        start=(k2 == 0), stop=(k2 == 1), perf_mode=DR)
                # bv is folded into bp on the host (softmax rows sum to 1 so
                # attn(v + bv) = attn(v) + bv exactly); eviction is a pure
                # cast, split vector/scalar to balance engine load
                if jp % 2 == 0:
                    nc.vector.tensor_copy(vS[:, jc * 4 + jp, :], pv[:, :])
                else:
                    nc.scalar.activation(
                        vS[:, jc * 4 + jp, :], pv[:, :], AF.Copy)

        with tc.tile_pool(name="pb_sb", bufs=3) as pb, \
             tc.tile_pool(name="pb_ps", bufs=6, space="PSUM") as pb_ps:
            for jc in range(JC):
                qkv_chunk(pb, pb_ps, jc)

        wrnd.release()    # free wq/wk/wv fp8 copies (LIFO with wstage)
        wstage.release()

        # ---------------- phase C: attention + projection + residual --------
        with tc.tile_pool(name="pc_sb", bufs=4) as pcs, \
             tc.tile_pool(name="pc_res", bufs=1) as pcr, \
             tc.tile_pool(name="pc_o", bufs=2) as pco, \
             tc.tile_pool(name="ps_s", bufs=2, space="PSUM") as ps_s, \
             tc.tile_pool(name="ps_o", bufs=1, space="PSUM") as ps_o, \
             tc.tile_pool(name="ps_r", bufs=1, space="PSUM") as ps_r, \
             tc.tile_pool(name="ps_y", bufs=1, space="PSUM") as ps_y:
            for ib in range(IB):
                po = ps_o.tile([128, CT, 512], F32)
                pr = ps_r.tile([1, 512], F32)
                # prefetch residual rows + bias for this i-block
                bpxs = []
                for ip in range(4):
                    xr = pcr.tile([128, C], F32, tag=f"xr{ip}")
                    nc.sync.dma_start(out=xr, in_=xqt[ib * 4 + ip, :, :])
                    bpx = pcr.tile([128, C], F32, tag=f"bpx{ip}")
                    nc.gpsimd.tensor_tensor(bpx[:, :], xr[:, :], bp_bc[:, :], OP.add)
                    bpxs.append(bpx)
                for j2 in range(JT // 2):
                    et = pcs.tile([128, 2, 512], F8, tag="exp")
                    for par in range(2):
                        j = 2 * j2 + par
                        pss = ps_s.tile([128, 512], F32, tag="scores")
                        for k2 in range(2):
                            nc.tensor.matmul(
                                pss[:, :], kT[:, 2 * k2:2 * k2 + 2, j * 128:(j + 1) * 128],
                                qT[:, 2 * k2:2 * k2 + 2, ib * 512:(ib + 1) * 512],
                                start=(k2 == 0), stop=(k2 == 1), perf_mode=DR)
                        nc.scalar.activation(et[:, par, :], pss[:, :], AF.Exp,
                                             bias=expb[:, :], scale=SM8)
                    for ct in range(CT):
                        nc.tensor.matmul(
                            po[:, ct, :], vS[:, 2 * j2:2 * j2 + 2, ct * 128:(ct + 1) * 128],
                            et[:, :, :], start=(j2 == 0), stop=(j2 == JT // 2 - 1),
                            perf_mode=DR)
                    # row-sums of exp: (2.0)^T @ etT -> [1, 512] (i on free dim)
                    nc.tensor.matmul(
                        pr[:, :], ones2[:, :, 0:1], et[:, :, :],
                        start=(j2 == 0), stop=(j2 == JT // 2 - 1), perf_mode=DR)
                # move the row-sums into per-partition layout [128, 4] via a
                # DRAM bounce (off-engine), then one cheap elementwise divide
                srow = pcs.tile([1, 512], F32, tag="srow")
                nc.scalar.activation(srow[:, :], pr[:, :], AF.Copy)
                nc.gpsimd.dma_start(out=sumscr[ib:ib + 1, :], in_=srow[:, :])
                st4 = pcr.tile([128, IB], F32, tag="st4")
                nc.gpsimd.dma_start(
                    out=st4[:, :],
                    in_=sumscr[ib, :].rearrange("(b p) -> p b", p=128))
                rt = pcr.tile([128, IB], F32, tag="rt")
                nc.vector.reciprocal(rt[:, :], st4[:, :])
                # unnormalized outT evicts on the scalar engine (scaled to fp8)
                ot = pco.tile([128, CT, 512], F8, tag="outT")
                for ct in range(CT):
                    nc.scalar.activation(ot[:, ct, :], po[:, ct, :], AF.Copy,
                                         scale=OTS)
                for ip in range(4):
                    py = ps_y.tile([128, 512], F32, tag="proj")
                    for c2 in range(2):
                        nc.tensor.matmul(
                            py[:, :], ot[:, 2 * c2:2 * c2 + 2, ip * 128:(ip + 1) * 128],
                            wpr[:, 2 * c2:2 * c2 + 2, :],
                            start=(c2 == 0), stop=(c2 == 1), perf_mode=DR)
                    y2 = pcs.tile([128, C], F32, tag="y2")
                    nc.vector.scalar_tensor_tensor(
                        y2[:, :], py[:, :], rt[:, ip:ip + 1], bpxs[ip][:, :],
                        OP.mult, OP.add)
                    nc.sync.dma_start(out=yt[ib * 4 + ip, :, :], in_=y2[:, :])

        cst.release()
        store.release()

    if split_waits:
        _split_multi_waits(nc)
    return nc


_PROGRAM = None


def _get_program():
    global _PROGRAM
    if _PROGRAM is None:
        _PROGRAM = build_program()
    return _PROGRAM


def make_in_maps(x, gamma, beta, wq, bq, wk, bk, wv, bv, wp, bp):
    import ml_dtypes
    f32 = lambda a: np.ascontiguousarray(a, dtype=np.float32)
    bf16 = lambda a: np.ascontiguousarray(np.asarray(a, dtype=np.float32).astype(ml_dtypes.bfloat16))
    xr = f32(x).reshape(B, HW, C)
    gsel = np.zeros((GROUPS, C), dtype=np.float32)
    for g in range(GROUPS):
        gsel[g, g * GSIZE:(g + 1) * GSIZE] = 1.0
    gsel2 = np.zeros((128, CT, GROUPS), dtype=np.float32)
    for p in range(128):
        for ct in range(CT):
            gsel2[p, ct, (ct * 128 + p) // GSIZE] = 1.0 / GSIZE
    common = {
        "wq": bf16(f32(wq) * WSC), "wk": bf16(f32(wk) * WSC),
        "wv": bf16(f32(wv) * WSC), "wp": bf16(f32(wp) * WSC),
        "bq": f32(bq).reshape(C, 1) * WSC, "bk": f32(bk).reshape(C, 1) * WSC,
        # bv rides through attention (softmax rows sum to 1): fold into bp
        "bp": f32(bp) + f32(bv) @ f32(wp),
        "gamma": f32(gamma).reshape(C, 1), "beta": f32(beta).reshape(C, 1),
        "gsel": gsel, "gsel2": gsel2,
    }
    in_maps = []
    for c in range(N_CORES):
        b, h = c // 2, c % 2
        m = dict(common)
        if h == 0:
            xp = xr[b]
        else:
            xp = np.concatenate([xr[b, HALF:], xr[b, :HALF]], axis=0)
        # pre-transpose to channel-major, chunk-tiled [JC, CT, 128, 512] bf16
        # so each chunk DMA is one contiguous 128KB read
        m["xT"] = np.ascontiguousarray(
            xp.T.astype(ml_dtypes.bfloat16).reshape(CT, 128, JC, 512)
            .transpose(2, 0, 1, 3))
        m["xq"] = np.ascontiguousarray(xr[b, h * HALF:(h + 1) * HALF])
        in_maps.append(m)
    return in_maps


def kernel(x, gamma, beta, wq, bq, wk, bk, wv, bv, wp, bp, _trace=False):
    nc = _get_program()
    in_maps = make_in_maps(x, gamma, beta, wq, bq, wk, bk, wv, bv, wp, bp)
    res = run_bass_kernel_spmd(nc, in_maps, list(range(N_CORES)), trace=_trace)
    out = np.empty((B, HW, C), dtype=np.float32)
    for c in range(N_CORES):
        b, h = c // 2, c % 2
        out[b, h * HALF:(h + 1) * HALF] = res.results[c]["y"]
    if _trace:
        kernel._last_result = res
    return out.reshape(B, H, W, C)


# revision 32
# speedup vs baseline: 1.8774x; 1.3571x over previous
"""AttentionBlock (GroupNorm + single-head 4096x4096 attention + proj + residual)
on 8 Trainium2 NeuronCores.

Sharding: core c = 2*b + h handles image b (of 4), query-half h (of 2).
Each core:
  - receives x pre-transposed to channel-major bf16 (host does the transpose),
  - computes GroupNorm statistics via bn_stats while x streams in,
  - computes kT [512,4096] and v [4096,512] for the full image (k/v duplicated
    across the half-pair, ~10% extra FLOPs, no collectives),
  - computes qT for its 2048 query rows,
  - attention over its 2048 queries, projection + bias + residual for its rows.

Precision: fp8e4m3 (TRN 240-max) with MatmulPerfMode.DoubleRow for ALL GEMMs
(hn/q/k/v/scores/PV/rowsum/proj), fp32 PSUM accumulation, fp32 GroupNorm
statistics, fp32 softmax row-sums / normalization, fp32 residual.

fp8 scale bookkeeping:
  - wq/wk/wv/wp host-scaled x16 (keeps N(0, 1/sqrt(C)) weights out of the fp8
    subnormal range); biases bq/bk/bv host-scaled x16 to match.
  - qT/kT hold 16q/16k; scores PSUM = 256*q.k; exp scale folds the 1/256.
  - exp has bias -2.0 (max score 6.81 -> et max e^4.81=123 < 240 fp8 max);
    the e^-2 factor cancels between numerator and row-sum.
  - vS holds 16v; po PSUM = 16*sum(et*v); ot eviction scales by 1/128.
  - proj PSUM py = (po/128) @ (16 wp) = 2*sum(et*v)@wp; rowsum matmul uses a
    2.0-valued ones vector so rt = 1/pr = 1/(2*sum(et)) normalizes exactly.
"""

import sys

sys.path.insert(0, "/opt/trn_rl_repo")

import numpy as np  # noqa: E402

import bass_rust  # noqa: E402
import concourse.bass as bass  # noqa: E402
import concourse.mybir as mybir  # noqa: E402
import concourse.tile as tile  # noqa: E402
from concourse.vector_clock import ScopedClock  # noqa: E402
from concourse.bass_utils import run_bass_kernel_spmd  # noqa: E402

F32 = mybir.dt.float32
BF16 = mybir.dt.bfloat16
F8 = mybir.dt.float8e4
AF = mybir.ActivationFunctionType
OP = mybir.AluOpType
DR = mybir.MatmulPerfMode.DoubleRow

B, H, W, C = 4, 64, 64, 512
HW = H * W            # 4096 positions per image
HALF = HW // 2        # 2048 query rows per core
GROUPS = 32
GSIZE = C // GROUPS   # 16 channels per group
EPS = 1e-5
N_CORES = 8
CT = C // 128         # 4 channel partition-tiles
JT = HW // 128        # 32 position partition-tiles
JC = HW // 512        # 8 position chunks (kT/v build)
QC = HALF // 512      # 4 query chunks (qT build)
IB = HALF // 512      # 4 query i-blocks (attention)
WSC = 16.0            # host-side weight/bias scale (fp8 subnormal avoidance)
SM8 = 1.0 / (WSC * WSC * float(np.sqrt(C)))   # exp scale on (16q).(16k) psum
EXPB = -2.0           # exp bias: keeps et = e^(s-2) <= e^4.9 < 240 (fp8 max)
OTS = 1.0 / 128.0     # po -> ot eviction scale (fp8 range)
ONESV = 2.0           # rowsum weights: pr = 2*sum(et) so rt=1/pr normalizes
                      # py = (po/128)@(16wp) = 2*sum(et*v)@wp exactly


# --- workaround: walrus in this container rejects instructions carrying more
# than one sync-wait command.  Move extra waits onto same-engine NOPs placed
# immediately before the instruction (engine program order makes this exact).
def _split_multi_waits(nc, max_waits=1):
    n = 0
    for f in nc.m.functions:
        for bb in f.blocks:
            newlist = []
            for inst in bb.instructions:
                si = inst.sync_info
                waits = list(si.on_wait) if si is not None else []
                if len(waits) > max_waits:
                    n += 1
                    for k, wt in enumerate(waits[:-max_waits]):
                        nop = bass_rust.InstNoOp(
                            name=f"{inst.name}-sw{k}", engine=inst.engine)
                        nop.sync_info = mybir.SyncInfo(on_wait=[wt], on_update=[])
                        newlist.append(nop)
                    inst.sync_info = mybir.SyncInfo(
                        on_wait=waits[-max_waits:], on_update=list(si.on_update))
                newlist.append(inst)
            bb.instructions[:] = newlist
    return n


def _split_drain_and_barrier(self, tick_clock, wait_clock):
    # same as TileContext._drain_and_barrier but with the tail drain's waits
    # split onto single-wait NOPs (same walrus limitation as above).
    drain_inst = self.nc.sync.drain()
    wait_clock.add_sem_waits(
        drain_inst.ins, ScopedClock({None: tick_clock.global_clock}))
    mi = drain_inst.ins
    waits = list(mi.sync_info.on_wait) if mi.sync_info is not None else []
    if len(waits) > 1:
        mi.sync_info.on_wait = []
        for wt in waits:
            wi = self.nc.sync.nop(nofuse=True, hint="tail_drain_wait")
            wi.ins.sync_info = mybir.SyncInfo(on_wait=[wt], on_update=[])
    self.nc.all_engine_barrier()
    assert self.sems is not None
    popped = self.nc._tile_sem_poison_stack.pop()
    assert popped is self._sem_poison
    self.nc.clear_and_free_semaphores(list(self.sems.allocated().values()))
    self.nc.all_engine_barrier()


tile.TileContext._drain_and_barrier = _split_drain_and_barrier


def build_program(split_waits=True):
    nc = bass.Bass()

    # xT rows are permuted per-core so the query half is always positions
    # [0, HALF), host-transposed to channel-major and slab-tiled
    # [2, CT, 128, 2048] bf16 so each (half, ct) slab DMA is one contiguous
    # 512KB read (DMA issue costs ~800ns of engine time -- few big DMAs win).
    xTd = nc.dram_tensor("xT", [2, CT, 128, 2048], BF16, kind="ExternalInput")
    xq = nc.dram_tensor("xq", [HALF, C], F32, kind="ExternalInput")
    wq = nc.dram_tensor("wq", [C, C], BF16, kind="ExternalInput")
    wk = nc.dram_tensor("wk", [C, C], BF16, kind="ExternalInput")
    wv = nc.dram_tensor("wv", [C, C], BF16, kind="ExternalInput")
    wp = nc.dram_tensor("wp", [C, C], BF16, kind="ExternalInput")
    # packed per-channel constants [128, CT, 4] = (16*bq, 16*bk, gamma, beta)
    cvecd = nc.dram_tensor("cvec", [128, CT, 4], F32, kind="ExternalInput")
    # bp here is host-computed bp + bv @ wp (bv folded through the attention)
    bpd = nc.dram_tensor("bp", [C], F32, kind="ExternalInput")
    gseld = nc.dram_tensor("gsel", [GROUPS, C], F32, kind="ExternalInput")
    # gsel2[p, ct, g] = 1/GSIZE where channel ct*128+p belongs to group g
    gsel2d = nc.dram_tensor("gsel2", [128, CT, GROUPS], F32, kind="ExternalInput")
    yd = nc.dram_tensor("y", [HALF, C], F32, kind="ExternalOutput")

    xqt = xq[:, :].rearrange("(t p) c -> t p c", p=128)   # [16,128,512]
    yt = yd[:, :].rearrange("(t p) c -> t p c", p=128)    # [16,128,512]

    with tile.TileContext(nc) as tc:
        # ---------------- persistent storage + constants ----------------
        store = tc.alloc_tile_pool(name="store", bufs=1)
        kT = store.tile([128, CT, HW], F8)       # kT[c%128, c//128, j] = 16k
        vS = store.tile([128, JT, C], F8)        # v[j%128, j//128, c] = 16v
        qT = store.tile([128, CT, HALF], F8)     # qT[c%128, c//128, i] = 16q
        # x^T in bf16, one tile per (channel-tile, image-half)
        xTs = [[store.tile([128, HALF], BF16, tag=f"xT{ct}_{hf}",
                           name=f"xT{ct}_{hf}") for hf in range(2)]
               for ct in range(CT)]
        wpr = store.tile([128, CT, C], F8)       # 16*wp, [cin%128, cin//128, cout]
        cst = tc.alloc_tile_pool(name="cst", bufs=1)
        gsel = cst.tile([GROUPS, C], F32)
        nc.scalar.dma_start(out=gsel, in_=gseld[:, :])
        gsel2 = cst.tile([128, CT, GROUPS], F32)
        nc.scalar.dma_start(out=gsel2, in_=gsel2d[:, :, :])
        # [128, 2, 16] so the DoubleRow weight AP's pair-dim step is 16 bytes
        # (walrus s3_lw_dual_fp8_restrictions requires step % 16 == 0)
        ones2 = cst.tile([128, 2, 16], F8)
        nc.vector.memset(ones2, ONESV)
        expb = cst.tile([128, 1], F32)
        nc.vector.memset(expb, EXPB)
        # DRAM scratch to re-layout softmax row-sums [1,512] -> [128,4]
        sumscr = nc.dram_tensor("sumscr", [IB, 512], F32)
        cv = cst.tile([128, CT, 4], F32)   # (16bq, 16bk, gamma, beta)
        nc.scalar.dma_start(out=cv, in_=cvecd[:, :, :])
        bp_bc = cst.tile([128, C], F32)
        nc.scalar.dma_start(out=bp_bc, in_=bpd[:].partition_broadcast(128))
        s_sb = cst.tile([128, CT], F32)   # GN scale per channel
        t_sb = cst.tile([128, CT], F32)   # GN shift per channel

        # 8 x-slab DMAs (512KB contiguous each) split over the sync/gpsimd
        # queues, issued FIRST (DMA issue costs ~800ns engine time each and
        # the slabs gate the GroupNorm stats chain).
        for hf in range(2):
            for ct in range(CT):
                xeng = nc.sync if ct < 2 else nc.gpsimd
                xeng.dma_start(out=xTs[ct][hf][:, :], in_=xTd[hf, ct, :, :])

        # weight cast bf16 -> fp8 (x16 applied on host); DMA on gpsimd queue
        # behind the x slabs; cast on the (fast) scalar engine -- Pool runs
        # fp8 casts ~5x below spec.
        wstage = tc.alloc_tile_pool(name="wstage", bufs=2)
        wrnd = tc.alloc_tile_pool(name="wrnd", bufs=1)
        wqr = wrnd.tile([128, CT, C], F8)
        wkr = wrnd.tile([128, CT, C], F8)
        wvr = wrnd.tile([128, CT, C], F8)
        for wd, wr in ((wq, wqr), (wk, wkr), (wv, wvr), (wp, wpr)):
            stg = wstage.tile([128, CT, C], BF16, tag="wstage")
            nc.gpsimd.dma_start(
                out=stg, in_=wd[:, :].rearrange("(t p) c -> p t c", p=128))
            nc.scalar.activation(wr[:, :, :], stg[:, :, :], AF.Copy)

        # ------- phase A: GroupNorm stats as the slabs land (no PE work) ----
        with tc.tile_pool(name="pa_small", bufs=1) as pas:
            stats_sb = pas.tile([128, CT, JC, 6], F32)
            for jc in range(JC):
                hf, sc = jc // 4, (jc % 4) * 512
                for ct in range(CT):
                    nc.vector.bn_stats(
                        out=stats_sb[:, ct, jc, :],
                        in_=xTs[ct][hf][:, sc:sc + 512])

            # per-channel stats -> per-group mean / E[x^2] (batched)
            with tc.tile_pool(name="pa_ps", bufs=2, space="PSUM") as pa_ps:
                g2 = pa_ps.tile([GROUPS, 2], F32, tag="gagg")
                mv_all = pas.tile([128, CT, 2], F32)
                sp_all = pas.tile([128, CT, 2], F32)
                for ct in range(CT):
                    nc.vector.bn_aggr(out=mv_all[:, ct, :], in_=stats_sb[:, ct, :, :])
                nc.vector.tensor_mul(sp_all[:, :, 0], mv_all[:, :, 0], mv_all[:, :, 0])
                nc.vector.tensor_add(sp_all[:, :, 1], sp_all[:, :, 0], mv_all[:, :, 1])
                nc.vector.tensor_copy(sp_all[:, :, 0], mv_all[:, :, 0])
                for ct in range(CT):
                    nc.tensor.matmul(g2[:, :], gsel2[:, ct, :], sp_all[:, ct, :],
                                     start=(ct == 0), stop=(ct == CT - 1))
                # group mean/var -> (mean, rstd)
                mv2 = pas.tile([GROUPS, 2], F32)
                nc.scalar.activation(mv2[:, :], g2[:, :], AF.Copy)   # (mean, E[x^2])
                var = pas.tile([GROUPS, 1], F32)
                nc.vector.tensor_mul(var[:, :], mv2[:, 0:1], mv2[:, 0:1])
                nc.vector.tensor_sub(var[:, :], mv2[:, 1:2], var[:, :])
                epst = pas.tile([GROUPS, 1], F32)
                nc.vector.memset(epst, EPS)
                sd = pas.tile([GROUPS, 1], F32)
                nc.scalar.activation(sd[:, :], var[:, :], AF.Sqrt, bias=epst[:, :])
                nc.vector.reciprocal(mv2[:, 1:2], sd[:, :])
                # broadcast group (mean, rstd) to channels, then s/t (batched)
                bc_all = pas.tile([128, CT, 2], F32)
                for ct in range(CT):
                    pbc = pa_ps.tile([128, 2], F32, tag="bcast")
                    nc.tensor.matmul(pbc[:, :], gsel[:, ct * 128:(ct + 1) * 128],
                                     mv2[:, :], start=True, stop=True)
                    nc.scalar.activation(bc_all[:, ct, :], pbc[:, :], AF.Copy)
                nc.vector.tensor_mul(s_sb[:, :], cv[:, :, 2], bc_all[:, :, 1])
                tmp = pas.tile([128, CT], F32)
                nc.vector.tensor_mul(tmp[:, :], bc_all[:, :, 0], s_sb[:, :])
                nc.vector.tensor_sub(t_sb[:, :], cv[:, :, 3], tmp[:, :])

        # ---------------- phase B: normalize + K,V (and Q) GEMMs ------------
        def qkv_chunk(pb, pb_ps, jc):
            hnT = pb.tile([128, CT, 512], F8, tag="hnT")
            for ct in range(CT):
                # hnT = s * xT + t  (per-channel; channels on partitions)
                nc.vector.tensor_scalar(
                    hnT[:, ct, :],
                    xTs[ct][jc // 4][:, (jc % 4) * 512:(jc % 4 + 1) * 512],
                    s_sb[:, ct:ct + 1], t_sb[:, ct:ct + 1], OP.mult, OP.add)
            for ct in range(CT):
                pk = pb_ps.tile([128, 512], F32, tag="qkv")
                for k2 in range(2):
                    nc.tensor.matmul(
                        pk[:, :], wkr[:, 2 * k2:2 * k2 + 2, ct * 128:(ct + 1) * 128],
                        hnT[:, 2 * k2:2 * k2 + 2, :],
                        start=(k2 == 0), stop=(k2 == 1), perf_mode=DR)
                nc.scalar.activation(
                    kT[:, ct, jc * 512:(jc + 1) * 512], pk[:, :],
                    AF.Identity, bias=cv[:, ct, 1:2])
            if jc < QC:   # rows [0, HALF) are the query rows
                for ct in range(CT):
                    pq = pb_ps.tile([128, 512], F32, tag="qkv")
                    for k2 in range(2):
                        nc.tensor.matmul(
                            pq[:, :], wqr[:, 2 * k2:2 * k2 + 2, ct * 128:(ct + 1) * 128],
                            hnT[:, 2 * k2:2 * k2 + 2, :],
                            start=(k2 == 0), stop=(k2 == 1), perf_mode=DR)
                    nc.scalar.activation(
                        qT[:, ct, jc * 512:(jc + 1) * 512], pq[:, :],
                        AF.Identity, bias=cv[:, ct, 0:1])
            for jp in range(4):
                pv = pb_ps.tile([128, 512], F32, tag="qkv")
                for k2 in range(2):
                    nc.tensor.matmul(
                        pv[:, :], hnT[:, 2 * k2:2 * k2 + 2, jp * 128:(jp + 1) * 128],
                        wvr[:, 2 * k2:2 * k2 + 2, :],
                        start=(k2 == 0), stop=(k2 == 1), perf_mode=DR)
                # bv is folded into bp on the host (softmax rows sum to 1 so
                # attn(v + bv) = attn(v) + bv exactly); eviction is a pure
                # cast, split vector/scalar to balance engine load
                if jp % 2 == 0:
                    nc.vector.tensor_copy(vS[:, jc * 4 + jp, :], pv[:, :])
                else:
                    nc.scalar.activation(
                        vS[:, jc * 4 + jp, :], pv[:, :], AF.Copy)

        with tc.tile_pool(name="pb_sb", bufs=3) as pb, \
             tc.tile_pool(name="pb_ps", bufs=6, space="PSUM") as pb_ps:
            for jc in range(JC):
                qkv_chunk(pb, pb_ps, jc)

        wrnd.release()    # free wq/wk/wv fp8 copies (LIFO with wstage)
        wstage.release()

        # ---------------- phase C: attention + projection + residual --------
        with tc.tile_pool(name="pc_sb", bufs=4) as pcs, \
             tc.tile_pool(name="pc_res", bufs=1) as pcr, \
             tc.tile_pool(name="pc_o", bufs=2) as pco, \
             tc.tile_pool(name="ps_s", bufs=2, space="PSUM") as ps_s, \
             tc.tile_pool(name="ps_o", bufs=1, space="PSUM") as ps_o, \
             tc.tile_pool(name="ps_r", bufs=1, space="PSUM") as ps_r, \
             tc.tile_pool(name="ps_y", bufs=1, space="PSUM") as ps_y:
            NP = JT // 2
            for ib in range(IB):
                po = ps_o.tile([128, CT, 512], F32)
                pr = ps_r.tile([1, 512], F32)
                # prefetch residual rows + bias for this i-block (one DMA)
                xrb = pcr.tile([128, 4, C], F32, tag="xrb")
                nc.sync.dma_start(
                    out=xrb,
                    in_=xq[ib * 512:(ib + 1) * 512, :].rearrange(
                        "(t p) c -> p t c", p=128))
                bpxs = []
                for ip in range(4):
                    bpx = pcr.tile([128, C], F32, tag=f"bpx{ip}")
                    nc.gpsimd.tensor_tensor(
                        bpx[:, :], xrb[:, ip, :], bp_bc[:, :], OP.add)
                    bpxs.append(bpx)

                # software-pipelined j-loop: emit exps(n) BEFORE pv(n-1) and
                # scores(n+1) so the exp's program-order semaphore threshold
                # does not include the PV matmuls (which stalled the PE by
                # ~0.4us per iteration otherwise).
                def scores(n):
                    pair = []
                    for par in range(2):
                        j = 2 * n + par
                        pss = ps_s.tile([128, 512], F32, tag="scores")
                        for k2 in range(2):
                            nc.tensor.matmul(
                                pss[:, :],
                                kT[:, 2 * k2:2 * k2 + 2, j * 128:(j + 1) * 128],
                                qT[:, 2 * k2:2 * k2 + 2, ib * 512:(ib + 1) * 512],
                                start=(k2 == 0), stop=(k2 == 1), perf_mode=DR)
                        pair.append(pss)
                    return pair

                def exps(n, pair):
                    et = pcs.tile([128, 2, 512], F8, tag="exp")
                    for par in range(2):
                        nc.scalar.activation(et[:, par, :], pair[par], AF.Exp,
                                             bias=expb[:, :], scale=SM8)
                    return et

                def pv(n, et):
                    for ct in range(CT):
                        nc.tensor.matmul(
                            po[:, ct, :],
                            vS[:, 2 * n:2 * n + 2, ct * 128:(ct + 1) * 128],
                            et[:, :, :], start=(n == 0), stop=(n == NP - 1),
                            perf_mode=DR)
                    # row-sums of exp: 2.0^T @ etT -> [1, 512] (i on free dim)
                    nc.tensor.matmul(
                        pr[:, :], ones2[:, :, 0:1], et[:, :, :],
                        start=(n == 0), stop=(n == NP - 1), perf_mode=DR)

                pair = scores(0)
                prev_et = None
                for n in range(NP):
                    et = exps(n, pair)
                    if n > 0:
                        pv(n - 1, prev_et)
                    if n + 1 < NP:
                        pair = scores(n + 1)
                    prev_et = et
                pv(NP - 1, prev_et)
                # move the row-sums into per-partition layout [128, 4] via a
                # DRAM bounce (off-engine), then one cheap elementwise divide
                srow = pcs.tile([1, 512], F32, tag="srow")
                nc.scalar.activation(srow[:, :], pr[:, :], AF.Copy)
                nc.gpsimd.dma_start(out=sumscr[ib:ib + 1, :], in_=srow[:, :])
                st4 = pcr.tile([128, IB], F32, tag="st4")
                nc.gpsimd.dma_start(
                    out=st4[:, :],
                    in_=sumscr[ib, :].rearrange("(b p) -> p b", p=128))
                rt = pcr.tile([128, IB], F32, tag="rt")
                nc.vector.reciprocal(rt[:, :], st4[:, :])
                # unnormalized outT evicts on the scalar engine (scaled to fp8)
                ot = pco.tile([128, CT, 512], F8, tag="outT")
                for ct in range(CT):
                    nc.scalar.activation(ot[:, ct, :], po[:, ct, :], AF.Copy,
                                         scale=OTS)
                for ip in range(4):
                    py = ps_y.tile([128, 512], F32, tag="proj")
                    for c2 in range(2):
                        nc.tensor.matmul(
                            py[:, :], ot[:, 2 * c2:2 * c2 + 2, ip * 128:(ip + 1) * 128],
                            wpr[:, 2 * c2:2 * c2 + 2, :],
                            start=(c2 == 0), stop=(c2 == 1), perf_mode=DR)
                    y2 = pcs.tile([128, C], F32, tag="y2")
                    nc.vector.scalar_tensor_tensor(
                        y2[:, :], py[:, :], rt[:, ip:ip + 1], bpxs[ip][:, :],
                        OP.mult, OP.add)
                    nc.sync.dma_start(out=yt[ib * 4 + ip, :, :], in_=y2[:, :])

        cst.release()
        store.release()

    if split_waits:
        _split_multi_waits(nc)
    return nc


_PROGRAM = None


def _get_program():
    global _PROGRAM
    if _PROGRAM is None:
        _PROGRAM = build_program()
    return _PROGRAM


def make_in_maps(x, gamma, beta, wq, bq, wk, bk, wv, bv, wp, bp):
    import ml_dtypes
    f32 = lambda a: np.ascontiguousarray(a, dtype=np.float32)
    bf16 = lambda a: np.ascontiguousarray(np.asarray(a, dtype=np.float32).astype(ml_dtypes.bfloat16))
    xr = f32(x).reshape(B, HW, C)
    gsel = np.zeros((GROUPS, C), dtype=np.float32)
    for g in range(GROUPS):
        gsel[g, g * GSIZE:(g + 1) * GSIZE] = 1.0
    gsel2 = np.zeros((128, CT, GROUPS), dtype=np.float32)
    for p in range(128):
        for ct in range(CT):
            gsel2[p, ct, (ct * 128 + p) // GSIZE] = 1.0 / GSIZE
    # packed per-channel constants: cvec[p, ct, :] = (16bq, 16bk, gamma, beta)
    cvec = np.stack([f32(bq) * WSC, f32(bk) * WSC, f32(gamma), f32(beta)],
                    axis=1).reshape(CT, 128, 4).transpose(1, 0, 2)
    common = {
        "wq": bf16(f32(wq) * WSC), "wk": bf16(f32(wk) * WSC),
        "wv": bf16(f32(wv) * WSC), "wp": bf16(f32(wp) * WSC),
        "cvec": np.ascontiguousarray(cvec),
        # bv rides through attention (softmax rows sum to 1): fold into bp
        "bp": f32(bp) + f32(bv) @ f32(wp),
        "gsel": gsel, "gsel2": gsel2,
    }
    in_maps = []
    for c in range(N_CORES):
        b, h = c // 2, c % 2
        m = dict(common)
        if h == 0:
            xp = xr[b]
        else:
            xp = np.concatenate([xr[b, HALF:], xr[b, :HALF]], axis=0)
        # pre-transpose to channel-major, slab-tiled [2, CT, 128, 2048] bf16
        # so each (half, ct) slab DMA is one contiguous 512KB read
        m["xT"] = np.ascontiguousarray(
            xp.T.astype(ml_dtypes.bfloat16).reshape(CT, 128, 2, 2048)
            .transpose(2, 0, 1, 3))
        m["xq"] = np.ascontiguousarray(xr[b, h * HALF:(h + 1) * HALF])
        in_maps.append(m)
    return in_maps


def kernel(x, gamma, beta, wq, bq, wk, bk, wv, bv, wp, bp, _trace=False):
    nc = _get_program()
    in_maps = make_in_maps(x, gamma, beta, wq, bq, wk, bk, wv, bv, wp, bp)
    res = run_bass_kernel_spmd(nc, in_maps, list(range(N_CORES)), trace=_trace)
    out = np.empty((B, HW, C), dtype=np.float32)
    for c in range(N_CORES):
        b, h = c // 2, c % 2
        out[b, h * HALF:(h + 1) * HALF] = res.results[c]["y"]
    if _trace:
        kernel._last_result = res
    return out.reshape(B, H, W, C)


# revision 40
# speedup vs baseline: 1.9065x; 1.0155x over previous
"""AttentionBlock (GroupNorm + single-head 4096x4096 attention + proj + residual)
on 8 Trainium2 NeuronCores.

Sharding: core c = 2*b + h handles image b (of 4), query-half h (of 2).
Each core:
  - receives x pre-transposed to channel-major bf16 (host does the transpose),
  - computes GroupNorm statistics via bn_stats while x streams in,
  - computes kT [512,4096] and v [4096,512] for the full image (k/v duplicated
    across the half-pair, ~10% extra FLOPs, no collectives),
  - computes qT for its 2048 query rows,
  - attention over its 2048 queries, projection + bias + residual for its rows.

Precision: fp8e4m3 (TRN 240-max) with MatmulPerfMode.DoubleRow for ALL GEMMs
(hn/q/k/v/scores/PV/rowsum/proj), fp32 PSUM accumulation, fp32 GroupNorm
statistics, fp32 softmax row-sums / normalization, fp32 residual.

fp8 scale bookkeeping:
  - wq/wk/wv/wp host-scaled x16 (keeps N(0, 1/sqrt(C)) weights out of the fp8
    subnormal range); biases bq/bk/bv host-scaled x16 to match.
  - qT/kT hold 16q/16k; scores PSUM = 256*q.k; exp scale folds the 1/256.
  - exp has bias -2.0 (max score 6.81 -> et max e^4.81=123 < 240 fp8 max);
    the e^-2 factor cancels between numerator and row-sum.
  - vS holds 16v; po PSUM = 16*sum(et*v); ot eviction scales by 1/128.
  - proj PSUM py = (po/128) @ (16 wp) = 2*sum(et*v)@wp; rowsum matmul uses a
    2.0-valued ones vector so rt = 1/pr = 1/(2*sum(et)) normalizes exactly.
"""

import sys

sys.path.insert(0, "/opt/trn_rl_repo")

import numpy as np  # noqa: E402

import bass_rust  # noqa: E402
import concourse.bass as bass  # noqa: E402
import concourse.mybir as mybir  # noqa: E402
import concourse.tile as tile  # noqa: E402
from concourse.vector_clock import ScopedClock  # noqa: E402
from concourse.bass_utils import run_bass_kernel_spmd  # noqa: E402

F32 = mybir.dt.float32
BF16 = mybir.dt.bfloat16
F8 = mybir.dt.float8e4
AF = mybir.ActivationFunctionType
OP = mybir.AluOpType
DR = mybir.MatmulPerfMode.DoubleRow

B, H, W, C = 4, 64, 64, 512
HW = H * W            # 4096 positions per image
HALF = HW // 2        # 2048 query rows per core
GROUPS = 32
GSIZE = C // GROUPS   # 16 channels per group
EPS = 1e-5
N_CORES = 8
CT = C // 128         # 4 channel partition-tiles
JT = HW // 128        # 32 position partition-tiles
JC = HW // 512        # 8 position chunks (kT/v build)
QC = HALF // 512      # 4 query chunks (qT build)
IB = HALF // 512      # 4 query i-blocks (attention)
WSC = 16.0            # host-side weight/bias scale (fp8 subnormal avoidance)
SM8 = 1.0 / (WSC * WSC * float(np.sqrt(C)))   # exp scale on (16q).(16k) psum
EXPB = -2.0           # exp bias: keeps et = e^(s-2) <= e^4.9 < 240 (fp8 max)
OTS = 1.0 / 128.0     # po -> ot eviction scale (fp8 range)
ONESV = 2.0           # rowsum weights: pr = 2*sum(et) so rt=1/pr normalizes
                      # py = (po/128)@(16wp) = 2*sum(et*v)@wp exactly


# --- workaround: walrus in this container rejects instructions carrying more
# than one sync-wait command.  Move extra waits onto same-engine NOPs placed
# immediately before the instruction (engine program order makes this exact).
def _split_multi_waits(nc, max_waits=1):
    n = 0
    for f in nc.m.functions:
        for bb in f.blocks:
            newlist = []
            for inst in bb.instructions:
                si = inst.sync_info
                waits = list(si.on_wait) if si is not None else []
                if len(waits) > max_waits:
                    n += 1
                    for k, wt in enumerate(waits[:-max_waits]):
                        nop = bass_rust.InstNoOp(
                            name=f"{inst.name}-sw{k}", engine=inst.engine)
                        nop.sync_info = mybir.SyncInfo(on_wait=[wt], on_update=[])
                        newlist.append(nop)
                    inst.sync_info = mybir.SyncInfo(
                        on_wait=waits[-max_waits:], on_update=list(si.on_update))
                newlist.append(inst)
            bb.instructions[:] = newlist
    return n


def _split_drain_and_barrier(self, tick_clock, wait_clock):
    # same as TileContext._drain_and_barrier but with the tail drain's waits
    # split onto single-wait NOPs (same walrus limitation as above).
    drain_inst = self.nc.sync.drain()
    wait_clock.add_sem_waits(
        drain_inst.ins, ScopedClock({None: tick_clock.global_clock}))
    mi = drain_inst.ins
    waits = list(mi.sync_info.on_wait) if mi.sync_info is not None else []
    if len(waits) > 1:
        mi.sync_info.on_wait = []
        for wt in waits:
            wi = self.nc.sync.nop(nofuse=True, hint="tail_drain_wait")
            wi.ins.sync_info = mybir.SyncInfo(on_wait=[wt], on_update=[])
    self.nc.all_engine_barrier()
    assert self.sems is not None
    popped = self.nc._tile_sem_poison_stack.pop()
    assert popped is self._sem_poison
    self.nc.clear_and_free_semaphores(list(self.sems.allocated().values()))
    self.nc.all_engine_barrier()


tile.TileContext._drain_and_barrier = _split_drain_and_barrier


def build_program(split_waits=True):
    nc = bass.Bass()

    # xT rows are permuted per-core so the query half is always positions
    # [0, HALF), host-transposed to channel-major and slab-tiled
    # [2, CT, 128, 2048] bf16 so each (half, ct) slab DMA is one contiguous
    # 512KB read (DMA issue costs ~800ns of engine time -- few big DMAs win).
    xTd = nc.dram_tensor("xT", [2, CT, 128, 2048], BF16, kind="ExternalInput")
    xq = nc.dram_tensor("xq", [HALF, C], F32, kind="ExternalInput")
    wq = nc.dram_tensor("wq", [C, C], BF16, kind="ExternalInput")
    wk = nc.dram_tensor("wk", [C, C], BF16, kind="ExternalInput")
    wv = nc.dram_tensor("wv", [C, C], BF16, kind="ExternalInput")
    wp = nc.dram_tensor("wp", [C, C], BF16, kind="ExternalInput")
    # packed per-channel constants [128, CT, 4] = (16*bq, 16*bk, gamma, beta)
    cvecd = nc.dram_tensor("cvec", [128, CT, 4], F32, kind="ExternalInput")
    # bp here is host-computed bp + bv @ wp (bv folded through the attention)
    bpd = nc.dram_tensor("bp", [C], F32, kind="ExternalInput")
    gseld = nc.dram_tensor("gsel", [GROUPS, C], F32, kind="ExternalInput")
    # gsel2[p, ct, g] = 1/GSIZE where channel ct*128+p belongs to group g
    gsel2d = nc.dram_tensor("gsel2", [128, CT, GROUPS], F32, kind="ExternalInput")
    yd = nc.dram_tensor("y", [HALF, C], F32, kind="ExternalOutput")

    xqt = xq[:, :].rearrange("(t p) c -> t p c", p=128)   # [16,128,512]
    yt = yd[:, :].rearrange("(t p) c -> t p c", p=128)    # [16,128,512]

    with tile.TileContext(nc) as tc:
        # ---------------- persistent storage + constants ----------------
        store = tc.alloc_tile_pool(name="store", bufs=1)
        kT = store.tile([128, CT, HW], F8)       # kT[c%128, c//128, j] = 16k
        vS = store.tile([128, JT, C], F8)        # v[j%128, j//128, c] = 16v
        qT = store.tile([128, CT, HALF], F8)     # qT[c%128, c//128, i] = 16q
        # x^T in bf16, one tile per (channel-tile, image-half)
        xTs = [[store.tile([128, HALF], BF16, tag=f"xT{ct}_{hf}",
                           name=f"xT{ct}_{hf}") for hf in range(2)]
               for ct in range(CT)]
        wpr = store.tile([128, CT, C], F8)       # 16*wp, [cin%128, cin//128, cout]
        cst = tc.alloc_tile_pool(name="cst", bufs=1)
        gsel = cst.tile([GROUPS, C], F32)
        nc.scalar.dma_start(out=gsel, in_=gseld[:, :])
        gsel2 = cst.tile([128, CT, GROUPS], F32)
        nc.scalar.dma_start(out=gsel2, in_=gsel2d[:, :, :])
        # [128, 2, 16] so the DoubleRow weight AP's pair-dim step is 16 bytes
        # (walrus s3_lw_dual_fp8_restrictions requires step % 16 == 0)
        ones2 = cst.tile([128, 2, 16], F8)
        nc.vector.memset(ones2, ONESV)
        expb = cst.tile([128, 1], F32)
        nc.vector.memset(expb, EXPB)
        # DRAM scratch to re-layout softmax row-sums [1,512] -> [128,4]
        sumscr = nc.dram_tensor("sumscr", [IB, 512], F32)
        cv = cst.tile([128, CT, 4], F32)   # (16bq, 16bk, gamma, beta)
        nc.scalar.dma_start(out=cv, in_=cvecd[:, :, :])
        bp_bc = cst.tile([128, C], F32)
        nc.scalar.dma_start(out=bp_bc, in_=bpd[:].partition_broadcast(128))
        s_sb = cst.tile([128, CT], F32)   # GN scale per channel
        t_sb = cst.tile([128, CT], F32)   # GN shift per channel

        # 8 x-slab DMAs (512KB contiguous each) split over the sync/gpsimd
        # queues, issued FIRST (DMA issue costs ~800ns engine time each and
        # the slabs gate the GroupNorm stats chain).
        for hf in range(2):
            for ct in range(CT):
                xeng = nc.sync if ct < 2 else nc.gpsimd
                xeng.dma_start(out=xTs[ct][hf][:, :], in_=xTd[hf, ct, :, :])

        # weight cast bf16 -> fp8 (x16 applied on host); DMA on gpsimd queue
        # behind the x slabs; cast on the (fast) scalar engine -- Pool runs
        # fp8 casts ~5x below spec.
        wstage = tc.alloc_tile_pool(name="wstage", bufs=2)
        wrnd = tc.alloc_tile_pool(name="wrnd", bufs=1)
        wqr = wrnd.tile([128, CT, C], F8)
        wkr = wrnd.tile([128, CT, C], F8)
        wvr = wrnd.tile([128, CT, C], F8)
        for wd, wr in ((wq, wqr), (wk, wkr), (wv, wvr), (wp, wpr)):
            stg = wstage.tile([128, CT, C], BF16, tag="wstage")
            nc.gpsimd.dma_start(
                out=stg, in_=wd[:, :].rearrange("(t p) c -> p t c", p=128))
            nc.scalar.activation(wr[:, :, :], stg[:, :, :], AF.Copy)

        # ------- phase A: GroupNorm stats as the slabs land (no PE work) ----
        with tc.tile_pool(name="pa_small", bufs=1) as pas:
            stats_sb = pas.tile([128, CT, JC, 6], F32)
            warmgate = pas.tile([128, 1], BF16)
            for jc in range(JC):
                hf, sc = jc // 4, (jc % 4) * 512
                for ct in range(CT):
                    nc.vector.bn_stats(
                        out=stats_sb[:, ct, jc, :],
                        in_=xTs[ct][hf][:, sc:sc + 512])
                if jc == 5:
                    # marker late in the stats chain; the PE warm-up matmuls
                    # below are gated on it so they run during the stats tail
                    # + GN scalar chain, flipping the HAM clock gate to 2.4GHz
                    # before phase B's real matmuls arrive.
                    nc.vector.tensor_copy(warmgate[:, :], stats_sb[:, 0, 0, 0:1])
            with tc.tile_pool(name="pa_warm", bufs=1, space="PSUM") as pwm:
                pw = pwm.tile([1, 512], F32)
                for r in range(24):
                    nc.tensor.matmul(
                        pw[:, :], warmgate[:, :], xTs[0][0][:, 0:512],
                        start=True, stop=True)

            # per-channel stats -> per-group mean / E[x^2] (batched)
            with tc.tile_pool(name="pa_ps", bufs=2, space="PSUM") as pa_ps:
                g2 = pa_ps.tile([GROUPS, 2], F32, tag="gagg")
                mv_all = pas.tile([128, CT, 2], F32)
                sp_all = pas.tile([128, CT, 2], F32)
                for ct in range(CT):
                    nc.vector.bn_aggr(out=mv_all[:, ct, :], in_=stats_sb[:, ct, :, :])
                nc.vector.tensor_mul(sp_all[:, :, 0], mv_all[:, :, 0], mv_all[:, :, 0])
                nc.vector.tensor_add(sp_all[:, :, 1], sp_all[:, :, 0], mv_all[:, :, 1])
                nc.vector.tensor_copy(sp_all[:, :, 0], mv_all[:, :, 0])
                for ct in range(CT):
                    nc.tensor.matmul(g2[:, :], gsel2[:, ct, :], sp_all[:, ct, :],
                                     start=(ct == 0), stop=(ct == CT - 1))
                # group mean/var -> (mean, rstd)
                mv2 = pas.tile([GROUPS, 2], F32)
                nc.scalar.activation(mv2[:, :], g2[:, :], AF.Copy)   # (mean, E[x^2])
                var = pas.tile([GROUPS, 1], F32)
                nc.vector.tensor_mul(var[:, :], mv2[:, 0:1], mv2[:, 0:1])
                nc.vector.tensor_sub(var[:, :], mv2[:, 1:2], var[:, :])
                epst = pas.tile([GROUPS, 1], F32)
                nc.vector.memset(epst, EPS)
                sd = pas.tile([GROUPS, 1], F32)
                nc.scalar.activation(sd[:, :], var[:, :], AF.Sqrt, bias=epst[:, :])
                nc.vector.reciprocal(mv2[:, 1:2], sd[:, :])
                # broadcast group (mean, rstd) to channels, then s/t (batched)
                bc_all = pas.tile([128, CT, 2], F32)
                for ct in range(CT):
                    pbc = pa_ps.tile([128, 2], F32, tag="bcast")
                    nc.tensor.matmul(pbc[:, :], gsel[:, ct * 128:(ct + 1) * 128],
                                     mv2[:, :], start=True, stop=True)
                    nc.scalar.activation(bc_all[:, ct, :], pbc[:, :], AF.Copy)
                nc.vector.tensor_mul(s_sb[:, :], cv[:, :, 2], bc_all[:, :, 1])
                tmp = pas.tile([128, CT], F32)
                nc.vector.tensor_mul(tmp[:, :], bc_all[:, :, 0], s_sb[:, :])
                nc.vector.tensor_sub(t_sb[:, :], cv[:, :, 3], tmp[:, :])

        # ---------------- phase B: normalize + K,V (and Q) GEMMs ------------
        def qkv_chunk(pb, pb_ps, jc):
            hnT = pb.tile([128, CT, 512], F8, tag="hnT")
            for ct in range(CT):
                # hnT = s * xT + t  (per-channel; channels on partitions)
                nc.vector.tensor_scalar(
                    hnT[:, ct, :],
                    xTs[ct][jc // 4][:, (jc % 4) * 512:(jc % 4 + 1) * 512],
                    s_sb[:, ct:ct + 1], t_sb[:, ct:ct + 1], OP.mult, OP.add)
            for ct in range(CT):
                pk = pb_ps.tile([128, 512], F32, tag="qkv")
                for k2 in range(2):
                    nc.tensor.matmul(
                        pk[:, :], wkr[:, 2 * k2:2 * k2 + 2, ct * 128:(ct + 1) * 128],
                        hnT[:, 2 * k2:2 * k2 + 2, :],
                        start=(k2 == 0), stop=(k2 == 1), perf_mode=DR)
                # kT eviction split scalar/vector (scalar is the busier engine)
                if ct % 2 == 0:
                    nc.scalar.activation(
                        kT[:, ct, jc * 512:(jc + 1) * 512], pk[:, :],
                        AF.Identity, bias=cv[:, ct, 1:2])
                else:
                    nc.vector.tensor_scalar(
                        kT[:, ct, jc * 512:(jc + 1) * 512], pk[:, :],
                        cv[:, ct, 1:2], None, OP.add)
            if jc < QC:   # rows [0, HALF) are the query rows
                for ct in range(CT):
                    pq = pb_ps.tile([128, 512], F32, tag="qkv")
                    for k2 in range(2):
                        nc.tensor.matmul(
                            pq[:, :], wqr[:, 2 * k2:2 * k2 + 2, ct * 128:(ct + 1) * 128],
                            hnT[:, 2 * k2:2 * k2 + 2, :],
                            start=(k2 == 0), stop=(k2 == 1), perf_mode=DR)
                    nc.scalar.activation(
                        qT[:, ct, jc * 512:(jc + 1) * 512], pq[:, :],
                        AF.Identity, bias=cv[:, ct, 0:1])
            for jp in range(4):
                pv = pb_ps.tile([128, 512], F32, tag="qkv")
                for k2 in range(2):
                    nc.tensor.matmul(
                        pv[:, :], hnT[:, 2 * k2:2 * k2 + 2, jp * 128:(jp + 1) * 128],
                        wvr[:, 2 * k2:2 * k2 + 2, :],
                        start=(k2 == 0), stop=(k2 == 1), perf_mode=DR)
                # bv is folded into bp on the host (softmax rows sum to 1 so
                # attn(v + bv) = attn(v) + bv exactly); eviction is a pure
                # cast, split vector/scalar to balance engine load
                if jp % 2 == 0:
                    nc.vector.tensor_copy(vS[:, jc * 4 + jp, :], pv[:, :])
                else:
                    nc.scalar.activation(
                        vS[:, jc * 4 + jp, :], pv[:, :], AF.Copy)

        with tc.tile_pool(name="pb_sb", bufs=3) as pb, \
             tc.tile_pool(name="pb_ps", bufs=6, space="PSUM") as pb_ps:
            for jc in range(JC):
                qkv_chunk(pb, pb_ps, jc)

        wrnd.release()    # free wq/wk/wv fp8 copies (LIFO with wstage)
        wstage.release()

        # ---------------- phase C: attention + projection + residual --------
        with tc.tile_pool(name="pc_sb", bufs=4) as pcs, \
             tc.tile_pool(name="pc_res", bufs=1) as pcr, \
             tc.tile_pool(name="pc_o", bufs=2) as pco, \
             tc.tile_pool(name="ps_s", bufs=2, space="PSUM") as ps_s, \
             tc.tile_pool(name="ps_o", bufs=1, space="PSUM") as ps_o, \
             tc.tile_pool(name="ps_r", bufs=1, space="PSUM") as ps_r, \
             tc.tile_pool(name="ps_y", bufs=1, space="PSUM") as ps_y:
            NP = JT // 2
            for ib in range(IB):
                po = ps_o.tile([128, CT, 512], F32)
                pr = ps_r.tile([1, 512], F32)
                # prefetch residual rows + bias for this i-block (one DMA)
                xrb = pcr.tile([128, 4, C], F32, tag="xrb")
                nc.sync.dma_start(
                    out=xrb,
                    in_=xq[ib * 512:(ib + 1) * 512, :].rearrange(
                        "(t p) c -> p t c", p=128))
                bpxs = []
                for ip in range(4):
                    bpx = pcr.tile([128, C], F32, tag=f"bpx{ip}")
                    nc.gpsimd.tensor_tensor(
                        bpx[:, :], xrb[:, ip, :], bp_bc[:, :], OP.add)
                    bpxs.append(bpx)

                # software-pipelined j-loop: emit exps(n) BEFORE pv(n-1) and
                # scores(n+1) so the exp's program-order semaphore threshold
                # does not include the PV matmuls (which stalled the PE by
                # ~0.4us per iteration otherwise).
                def scores(n):
                    pair = []
                    for par in range(2):
                        j = 2 * n + par
                        pss = ps_s.tile([128, 512], F32, tag="scores")
                        for k2 in range(2):
                            nc.tensor.matmul(
                                pss[:, :],
                                kT[:, 2 * k2:2 * k2 + 2, j * 128:(j + 1) * 128],
                                qT[:, 2 * k2:2 * k2 + 2, ib * 512:(ib + 1) * 512],
                                start=(k2 == 0), stop=(k2 == 1), perf_mode=DR)
                        pair.append(pss)
                    return pair

                def exps(n, pair):
                    et = pcs.tile([128, 2, 512], F8, tag="exp")
                    for par in range(2):
                        nc.scalar.activation(et[:, par, :], pair[par], AF.Exp,
                                             bias=expb[:, :], scale=SM8)
                    return et

                def pv(n, et):
                    for ct in range(CT):
                        nc.tensor.matmul(
                            po[:, ct, :],
                            vS[:, 2 * n:2 * n + 2, ct * 128:(ct + 1) * 128],
                            et[:, :, :], start=(n == 0), stop=(n == NP - 1),
                            perf_mode=DR)
                    # row-sums of exp: 2.0^T @ etT -> [1, 512] (i on free dim)
                    nc.tensor.matmul(
                        pr[:, :], ones2[:, :, 0:1], et[:, :, :],
                        start=(n == 0), stop=(n == NP - 1), perf_mode=DR)

                pair = scores(0)
                prev_et = None
                for n in range(NP):
                    et = exps(n, pair)
                    if n > 0:
                        pv(n - 1, prev_et)
                    if n + 1 < NP:
                        pair = scores(n + 1)
                    prev_et = et
                pv(NP - 1, prev_et)
                # move the row-sums into per-partition layout [128, 4] via a
                # DRAM bounce (off-engine), then one cheap elementwise divide
                srow = pcs.tile([1, 512], F32, tag="srow")
                nc.scalar.activation(srow[:, :], pr[:, :], AF.Copy)
                nc.gpsimd.dma_start(out=sumscr[ib:ib + 1, :], in_=srow[:, :])
                st4 = pcr.tile([128, IB], F32, tag="st4")
                nc.gpsimd.dma_start(
                    out=st4[:, :],
                    in_=sumscr[ib, :].rearrange("(b p) -> p b", p=128))
                rt = pcr.tile([128, IB], F32, tag="rt")
                nc.vector.reciprocal(rt[:, :], st4[:, :])
                # unnormalized outT eviction (scaled into fp8 range), split
                # scalar/vector so neither engine gates the projection
                ot = pco.tile([128, CT, 512], F8, tag="outT")
                for ct in range(CT):
                    if ct % 2 == 0:
                        nc.scalar.activation(ot[:, ct, :], po[:, ct, :],
                                             AF.Copy, scale=OTS)
                    else:
                        nc.vector.tensor_scalar(ot[:, ct, :], po[:, ct, :],
                                                OTS, None, OP.mult)
                for ip in range(4):
                    py = ps_y.tile([128, 512], F32, tag="proj")
                    for c2 in range(2):
                        nc.tensor.matmul(
                            py[:, :], ot[:, 2 * c2:2 * c2 + 2, ip * 128:(ip + 1) * 128],
                            wpr[:, 2 * c2:2 * c2 + 2, :],
                            start=(c2 == 0), stop=(c2 == 1), perf_mode=DR)
                    y2 = pcs.tile([128, C], F32, tag="y2")
                    nc.vector.scalar_tensor_tensor(
                        y2[:, :], py[:, :], rt[:, ip:ip + 1], bpxs[ip][:, :],
                        OP.mult, OP.add)
                    nc.sync.dma_start(out=yt[ib * 4 + ip, :, :], in_=y2[:, :])

        cst.release()
        store.release()

    if split_waits:
        _split_multi_waits(nc)
    return nc


_PROGRAM = None


def _get_program():
    global _PROGRAM
    if _PROGRAM is None:
        _PROGRAM = build_program()
    return _PROGRAM


def make_in_maps(x, gamma, beta, wq, bq, wk, bk, wv, bv, wp, bp):
    import ml_dtypes
    f32 = lambda a: np.ascontiguousarray(a, dtype=np.float32)
    bf16 = lambda a: np.ascontiguousarray(np.asarray(a, dtype=np.float32).astype(ml_dtypes.bfloat16))
    xr = f32(x).reshape(B, HW, C)
    gsel = np.zeros((GROUPS, C), dtype=np.float32)
    for g in range(GROUPS):
        gsel[g, g * GSIZE:(g + 1) * GSIZE] = 1.0
    gsel2 = np.zeros((128, CT, GROUPS), dtype=np.float32)
    for p in range(128):
        for ct in range(CT):
            gsel2[p, ct, (ct * 128 + p) // GSIZE] = 1.0 / GSIZE
    # packed per-channel constants: cvec[p, ct, :] = (16bq, 16bk, gamma, beta)
    cvec = np.stack([f32(bq) * WSC, f32(bk) * WSC, f32(gamma), f32(beta)],
                    axis=1).reshape(CT, 128, 4).transpose(1, 0, 2)
    common = {
        "wq": bf16(f32(wq) * WSC), "wk": bf16(f32(wk) * WSC),
        "wv": bf16(f32(wv) * WSC), "wp": bf16(f32(wp) * WSC),
        "cvec": np.ascontiguousarray(cvec),
        # bv rides through attention (softmax rows sum to 1): fold into bp
        "bp": f32(bp) + f32(bv) @ f32(wp),
        "gsel": gsel, "gsel2": gsel2,
    }
    in_maps = []
    for c in range(N_CORES):
        b, h = c // 2, c % 2
        m = dict(common)
        if h == 0:
            xp = xr[b]
        else:
            xp = np.concatenate([xr[b, HALF:], xr[b, :HALF]], axis=0)
        # pre-transpose to channel-major, slab-tiled [2, CT, 128, 2048] bf16
        # so each (half, ct) slab DMA is one contiguous 512KB read
        m["xT"] = np.ascontiguousarray(
            xp.T.astype(ml_dtypes.bfloat16).reshape(CT, 128, 2, 2048)
            .transpose(2, 0, 1, 3))
        m["xq"] = np.ascontiguousarray(xr[b, h * HALF:(h + 1) * HALF])
        in_maps.append(m)
    return in_maps


def kernel(x, gamma, beta, wq, bq, wk, bk, wv, bv, wp, bp, _trace=False):
    nc = _get_program()
    in_maps = make_in_maps(x, gamma, beta, wq, bq, wk, bk, wv, bv, wp, bp)
    res = run_bass_kernel_spmd(nc, in_maps, list(range(N_CORES)), trace=_trace)
    out = np.empty((B, HW, C), dtype=np.float32)
    for c in range(N_CORES):
        b, h = c // 2, c % 2
        out[b, h * HALF:(h + 1) * HALF] = res.results[c]["y"]
    if _trace:
        kernel._last_result = res
    return out.reshape(B, H, W, C)
